# revision 1
# baseline (speedup 1.0000x reference)
"""CGNN layer kernel for Trainium2 (8 NeuronCores, SPMD).

Sharding: core c owns batch b = c//2 and receiver-node half i0 = (c%2)*128.
Each core computes its (128, 128) output shard from full-j message passing.

Math (per core, b fixed):
  z[i,j,:]  = W1a x_i + W1b x_j + W1d a_ij + W1c c + b1        (pre-activation)
  s[i,:]    = sum_j mask_j * silu(z[i,j,:])
  aggr      = W2 s + b2 * (#live j)
  u         = silu(W3 [x, aggr] + b3); out = LN(x + W4 u + b4) * gamma + beta

Device layout: z kept as (h=128 partitions, j=256 free) per receiver i.
  - adj term: PE-transpose 4-receiver stacks of adj (j,r)->(r,j), masked evict,
    then K=32 row-tiled matmuls (tile_position) against replicated W1d^T.
  - x_j term: one K=128 matmul vs pre-masked x^T (same operands every i).
  - bias+silu+sum_j: single ACT op (bias port + accum_out).
  - masked-j bias pollution removed in closed form: s -= nm0 * silu(beta_i).

Scheduling notes: walrus gives compute instructions a budget of ONE semaphore
wait, and only waits arising from real data dependencies update Tile's
per-engine clock. The kernel therefore "absorbs" cross-engine production ticks
with tiny 1x1 matmuls that genuinely read one stale element of the producer
tile (into a dedicated PSUM scratch column), so every real matmul needs at
most its single PSUM-recycle wait. All MLP biases are folded into PSUM via
K=1 rank-1 matmuls of host-provided bias ROWS against a ones row, so no ACT
instruction ever waits on a DMA. All PSUM pools live for the whole program so
banks never alias across phases.
"""

import numpy as np
import ml_dtypes
ml_bf16 = ml_dtypes.bfloat16
from contextlib import ExitStack

import concourse.bass as bass
import concourse.bacc as bacc
import concourse.mybir as mybir
import concourse.tile as tile
from concourse.bass_utils import run_bass_kernel_spmd
from concourse.tile_rust import add_dep_helper

B, N, H, R = 4, 256, 128, 32
NI = 128          # receivers per core
NQ = NI // 4      # receiver quads
FP = mybir.dt.float32
BF = mybir.dt.bfloat16
EPS = 1e-5
ALU = mybir.AluOpType
ACTF = mybir.ActivationFunctionType

_cache = {}


def _order(later, earlier):
    a = later.ins if hasattr(later, "ins") else later
    b = earlier.ins if hasattr(earlier, "ins") else earlier
    add_dep_helper(a, b, sync=False, reason="pe order")


def _build_program():
    nc = bacc.Bacc()

    # ---- per-core DRAM parameters ----
    adj = nc.declare_dram_parameter("adj", [NI, N, R], FP, isOutput=False)
    x_all = nc.declare_dram_parameter("x_all", [N, H], FP, isOutput=False)
    xi = nc.declare_dram_parameter("xi", [NI, H], FP, isOutput=False)
    maskf = nc.declare_dram_parameter("maskf", [N], FP, isOutput=False)
    condrep = nc.declare_dram_parameter("condrep", [2 * H, H], FP, isOutput=False)
    w1aT = nc.declare_dram_parameter("w1aT", [H, H], FP, isOutput=False)
    w1bT = nc.declare_dram_parameter("w1bT", [H, H], BF, isOutput=False)
    w1cT = nc.declare_dram_parameter("w1cT", [2 * H, H], FP, isOutput=False)
    w1dTrep = nc.declare_dram_parameter("w1dTrep", [H, H], BF, isOutput=False)
    w2T = nc.declare_dram_parameter("w2T", [H, H], FP, isOutput=False)
    w3aT = nc.declare_dram_parameter("w3aT", [H, H], FP, isOutput=False)
    w3bT = nc.declare_dram_parameter("w3bT", [H, H], FP, isOutput=False)
    w4T = nc.declare_dram_parameter("w4T", [H, H], FP, isOutput=False)
    b1row = nc.declare_dram_parameter("b1row", [1, H], FP, isOutput=False)
    b2row = nc.declare_dram_parameter("b2row", [1, H], FP, isOutput=False)
    b3row = nc.declare_dram_parameter("b3row", [1, H], FP, isOutput=False)
    b4row = nc.declare_dram_parameter("b4row", [1, H], FP, isOutput=False)
    onesrow = nc.declare_dram_parameter("onesrow", [1, NI], FP, isOutput=False)
    identp = nc.declare_dram_parameter("identp", [H, H], FP, isOutput=False)
    gamma_rep = nc.declare_dram_parameter("gamma_rep", [H, H], FP, isOutput=False)
    beta_rep = nc.declare_dram_parameter("beta_rep", [H, H], FP, isOutput=False)
    out = nc.declare_dram_parameter("out", [NI, H], FP, isOutput=True)

    with ExitStack() as ctx:
        tc = ctx.enter_context(tile.TileContext(nc))
        const = ctx.enter_context(tc.tile_pool(name="const", bufs=1))
        persist = ctx.enter_context(tc.tile_pool(name="persist", bufs=1))
        work = ctx.enter_context(tc.tile_pool(name="work", bufs=2))
        adjbuf = ctx.enter_context(tc.tile_pool(name="adjbuf", bufs=3))
        scr = ctx.enter_context(tc.tile_pool(name="scr", bufs=3))
        # PSUM: 2 (setup/epilogue) + 4 (z) + 2 (adjT)
        pep = ctx.enter_context(tc.tile_pool(name="pep", bufs=2, space="PSUM"))
        pz = ctx.enter_context(tc.tile_pool(name="pz", bufs=4, space="PSUM"))
        pt = ctx.enter_context(tc.tile_pool(name="pt", bufs=2, space="PSUM"))

        cload_tiles = []

        def cload(ap, shape, tag, dt=FP):
            if not isinstance(ap, bass.AP):
                ap = ap[:]
            t = const.tile(shape, dt, tag=tag, name=tag)
            nc.sync.dma_start(out=t, in_=ap)
            cload_tiles.append(t)
            return t

        ident_sb = cload(identp, [H, H], "ident")
        w1aT_sb = cload(w1aT, [H, H], "w1aT")
        w1bT_sb = cload(w1bT, [H, H], "w1bT", dt=BF)
        w1cT_sb0 = cload(w1cT[0:H, :], [H, H], "w1cT0")
        w1cT_sb1 = cload(w1cT[H:2 * H, :], [H, H], "w1cT1")
        w1dTrep_sb = cload(w1dTrep, [H, H], "w1dTrep", dt=BF)
        w2T_sb = cload(w2T, [H, H], "w2T")
        w3aT_sb = cload(w3aT, [H, H], "w3aT")
        w3bT_sb = cload(w3bT, [H, H], "w3bT")
        w4T_sb = cload(w4T, [H, H], "w4T")
        condrep_sb0 = cload(condrep[0:H, :], [H, H], "condrep0")
        condrep_sb1 = cload(condrep[H:2 * H, :], [H, H], "condrep1")
        b1r_sb = cload(b1row, [1, H], "b1r")
        b2r_sb = cload(b2row, [1, H], "b2r")
        b3r_sb = cload(b3row, [1, H], "b3r")
        b4r_sb = cload(b4row, [1, H], "b4r")
        ones_sb = cload(onesrow, [1, NI], "onesr")
        xi_sb = cload(xi, [NI, H], "xi")
        xall_sb0 = cload(x_all[0:H, :], [H, H], "xall0")
        xall_sb1 = cload(x_all[H:N, :], [H, H], "xall1")
        gamma_sb = cload(gamma_rep, [H, H], "gamma_rep")
        beta_sb = cload(beta_rep, [H, H], "beta_rep")

        # mask broadcast to all partitions: (128, 256)
        maskrep = persist.tile([H, N], FP, tag="maskrep", name="maskrep")
        maskf_ap = maskf[:]
        mask_bcast = bass.AP(tensor=maskf_ap.tensor, offset=maskf_ap.offset,
                             ap=[[0, H]] + list(maskf_ap.ap))
        nc.sync.dma_start(out=maskrep, in_=mask_bcast)

        # per-partition live-count and masked-out-count of senders
        msum = persist.tile([H, 1], FP, tag="msum", name="msum")
        mrow_scr = persist.tile([H, N], FP, tag="mrow_scr", name="mrow_scr")
        nc.vector.tensor_scalar(mrow_scr, maskrep, 1.0, None,
                                ALU.mult, ALU.add, accum_out=msum)
        nm0col = persist.tile([H, 1], FP, tag="nm0col", name="nm0col")
        nc.vector.tensor_scalar(nm0col, msum, -1.0, float(N), ALU.mult, ALU.add)
        # msum replicated as a row (all partitions of msum hold the same value)
        msum_row = persist.tile([1, NI], FP, tag="msum_row", name="msum_row")
        nc.vector.tensor_scalar(msum_row, ones_sb, msum[0:1, 0:1], None,
                                ALU.mult)

        xTm = persist.tile([H, N], BF, tag="xTm", name="xTm")
        xTi = persist.tile([H, NI], FP, tag="xTi", name="xTi")
        ACb = persist.tile([H, NI], FP, tag="ACb", name="ACb")
        siluAC = persist.tile([H, NI], FP, tag="siluAC", name="siluAC")
        korr = persist.tile([H, NI], FP, tag="korr", name="korr")
        S_raw = persist.tile([H, NI], FP, tag="S_raw", name="S_raw")

        # ---- setup: x transposes, ACb ----
        for half, xall_h in ((0, xall_sb0), (1, xall_sb1)):
            pxt = pep.tile([H, H], FP, tag="ps", name="pxt")
            nc.tensor.transpose(pxt, xall_h, ident_sb)
            nc.vector.scalar_tensor_tensor(
                out=xTm[:, half * H:(half + 1) * H], in0=pxt, scalar=1.0,
                in1=maskrep[:, half * H:(half + 1) * H],
                op0=ALU.mult, op1=ALU.mult)

        pxi = pep.tile([H, H], FP, tag="ps", name="pxi")
        nc.tensor.transpose(pxi, xi_sb, ident_sb)
        nc.vector.tensor_copy(xTi, pxi)

        # ACb = W1a x_i + W1c c + b1  -> (128 h, 128 i)
        pA = pep.tile([H, NI], FP, tag="ps", name="pA")
        nc.tensor.matmul(pA, lhsT=w1aT_sb, rhs=xTi, start=True, stop=False)
        nc.tensor.matmul(pA, lhsT=w1cT_sb0, rhs=condrep_sb0,
                         start=False, stop=False)
        nc.tensor.matmul(pA, lhsT=w1cT_sb1, rhs=condrep_sb1,
                         start=False, stop=False)
        nc.tensor.matmul(pA, lhsT=b1r_sb, rhs=ones_sb,
                         start=False, stop=True)
        nc.scalar.activation(ACb, pA, ACTF.Copy)

        # korr[h,i] = nm0 * silu(ACb[h,i])
        nc.scalar.activation(siluAC, ACb, ACTF.Silu)
        nc.vector.tensor_scalar(korr, siluAC, nm0col, None, ALU.mult)

        # ---- main loop over receiver quads ----
        stacks = persist.tile([H, NQ, 2, 4, R], FP, tag="stacks",
                              name="stacks")
        for q in range(NQ):
            st0 = stacks[:, q, 0]
            st1 = stacks[:, q, 1]
            for jt, st, eng in ((0, st0, nc.sync), (1, st1, nc.scalar)):
                asrc = adj[4 * q:4 * q + 4, jt * H:(jt + 1) * H, :]
                eng.dma_start(out=st, in_=asrc.rearrange("g j r -> j g r"))

            ptile = pt.tile([H, N], FP, tag="ptile", name="ptile")
            nc.tensor.transpose(
                ptile[:, 0:H], st0.rearrange("j g r -> j (g r)"), ident_sb)
            nc.tensor.transpose(
                ptile[:, H:N], st1.rearrange("j g r -> j (g r)"), ident_sb)

            atile = adjbuf.tile([H, N], BF, tag="atile", name="atile")
            nc.vector.scalar_tensor_tensor(
                out=atile, in0=ptile, scalar=1.0, in1=maskrep,
                op0=ALU.mult, op1=ALU.mult)

            zts = []
            for g in range(4):
                zt = pz.tile([H, N], FP, tag="zt", name="zt")
                nc.tensor.matmul(zt, lhsT=w1bT_sb, rhs=xTm,
                                 start=True, stop=False)
                zts.append(zt)
            for g in range(4):
                nc.tensor.matmul(
                    zts[g], lhsT=w1dTrep_sb[32 * g:32 * g + 32, :],
                    rhs=atile[32 * g:32 * g + 32, :],
                    start=False, stop=True, tile_position=(32 * g, 0))
            for g in range(4):
                li = 4 * q + g
                sct = scr.tile([H, N], BF, tag="sct", name="sct")
                nc.scalar.activation(sct, zts[g], ACTF.Silu,
                                     bias=ACb[:, li:li + 1])
                sink = scr.tile([H, N], BF, tag="sink", name="sink")
                nc.vector.tensor_scalar(sink, sct, 1.0, None, ALU.mult,
                                        ALU.add, accum_out=S_raw[:, li:li + 1])

        # ---- epilogue ----
        S_true = persist.tile([H, NI], FP, tag="S_true", name="S_true")
        nc.vector.scalar_tensor_tensor(out=S_true, in0=S_raw, scalar=0.0,
                                       in1=korr, op0=ALU.add,
                                       op1=ALU.subtract)
        # aggr = W2 s + b2 * live_count
        pa = pep.tile([H, NI], FP, tag="ps", name="pa")
        nc.tensor.matmul(pa, lhsT=w2T_sb, rhs=S_true, start=True, stop=False)
        nc.tensor.matmul(pa, lhsT=b2r_sb, rhs=msum_row, start=False,
                         stop=True)
        aggrT = work.tile([H, NI], FP, tag="aggrT", name="aggrT")
        nc.scalar.activation(aggrT, pa, ACTF.Copy)

        pu = pep.tile([H, NI], FP, tag="ps", name="pu")
        nc.tensor.matmul(pu, lhsT=w3aT_sb, rhs=xTi, start=True, stop=False)
        nc.tensor.matmul(pu, lhsT=w3bT_sb, rhs=aggrT, start=False, stop=False)
        nc.tensor.matmul(pu, lhsT=b3r_sb, rhs=ones_sb, start=False,
                         stop=True)
        u_sb = work.tile([H, NI], FP, tag="u_sb", name="u_sb")
        nc.scalar.activation(u_sb, pu, ACTF.Silu)

        pupd = pep.tile([H, NI], FP, tag="ps", name="pupd")
        nc.tensor.matmul(pupd, lhsT=w4T_sb, rhs=u_sb, start=True, stop=False)
        nc.tensor.matmul(pupd, lhsT=b4r_sb, rhs=ones_sb, start=False,
                         stop=True)
        updT = work.tile([H, NI], FP, tag="updT", name="updT")
        nc.scalar.activation(updT, pupd, ACTF.Copy)

        py = pep.tile([NI, H], FP, tag="ps", name="py")
        nc.tensor.transpose(py, updT, ident_sb)

        y_sb = work.tile([NI, H], FP, tag="y_sb", name="y_sb")
        rowsum = work.tile([NI, 1], FP, tag="rowsum", name="rowsum")
        nc.vector.scalar_tensor_tensor(out=y_sb, in0=py, scalar=0.0,
                                       in1=xi_sb, op0=ALU.add, op1=ALU.add,
                                       accum_out=rowsum)
        negmu = work.tile([NI, 1], FP, tag="negmu", name="negmu")
        nc.vector.tensor_scalar(negmu, rowsum, -1.0 / H, None, ALU.mult)

        ysq = work.tile([NI, H], FP, tag="ysq", name="ysq")
        sumsq = work.tile([NI, 1], FP, tag="sumsq", name="sumsq")
        nc.vector.scalar_tensor_tensor(out=ysq, in0=y_sb, scalar=0.0,
                                       in1=y_sb, op0=ALU.add, op1=ALU.mult,
                                       accum_out=sumsq)
        # var + eps = sumsq/H - mu^2 + eps
        ex2 = work.tile([NI, 1], FP, tag="ex2", name="ex2")
        nc.vector.tensor_scalar(ex2, sumsq, 1.0 / H, float(EPS),
                                ALU.mult, ALU.add)
        musq = work.tile([NI, 1], FP, tag="musq", name="musq")
        nc.vector.scalar_tensor_tensor(out=musq, in0=negmu, scalar=0.0,
                                       in1=negmu, op0=ALU.add, op1=ALU.mult)
        vare = work.tile([NI, 1], FP, tag="vare", name="vare")
        nc.vector.scalar_tensor_tensor(out=vare, in0=ex2, scalar=0.0,
                                       in1=musq, op0=ALU.add,
                                       op1=ALU.subtract)
        sd = work.tile([NI, 1], FP, tag="sd", name="sd")
        nc.scalar.activation(sd, vare, ACTF.Sqrt)
        rstd = work.tile([NI, 1], FP, tag="rstd", name="rstd")
        nc.vector.reciprocal(rstd, sd)

        yn = work.tile([NI, H], FP, tag="yn", name="yn")
        nc.vector.tensor_scalar(yn, y_sb, negmu, rstd, ALU.add, ALU.mult)
        yg = work.tile([NI, H], FP, tag="yg", name="yg")
        nc.vector.scalar_tensor_tensor(out=yg, in0=yn, scalar=0.0,
                                       in1=gamma_sb, op0=ALU.add,
                                       op1=ALU.mult)
        yfin = work.tile([NI, H], FP, tag="yfin", name="yfin")
        nc.vector.scalar_tensor_tensor(out=yfin, in0=yg, scalar=0.0,
                                       in1=beta_sb, op0=ALU.add,
                                       op1=ALU.add)
        nc.sync.dma_start(out=out[:], in_=yfin)

    nc.finalize()
    return nc


def _get_program():
    if "nc" not in _cache:
        _cache["nc"] = _build_program()
    return _cache["nc"]


def kernel(x, adj_dist, mask, cond_vec, W1, b1, W2, b2, W3, b3, W4, b4,
           gamma, beta):
    x = np.asarray(x, dtype=np.float32)
    adj_dist = np.asarray(adj_dist, dtype=np.float32)
    mask_np = np.asarray(mask)
    cond_vec = np.asarray(cond_vec, dtype=np.float32)
    W1 = np.asarray(W1, dtype=np.float32)
    W2 = np.asarray(W2, dtype=np.float32)
    W3 = np.asarray(W3, dtype=np.float32)
    W4 = np.asarray(W4, dtype=np.float32)

    def c(a):
        return np.ascontiguousarray(a, dtype=np.float32)

    shared = dict(
        w1aT=c(W1[:, 0:H].T),
        w1bT=np.ascontiguousarray(W1[:, H:2 * H].T.astype(ml_bf16)),
        w1cT=c(W1[:, 2 * H + R:].T),
        w1dTrep=np.ascontiguousarray(
            np.tile(W1[:, 2 * H:2 * H + R].T, (4, 1)).astype(ml_bf16)),
        w2T=c(W2.T), w3aT=c(W3[:, 0:H].T), w3bT=c(W3[:, H:2 * H].T),
        w4T=c(W4.T),
        b1row=c(np.asarray(b1).reshape(1, H)),
        b2row=c(np.asarray(b2).reshape(1, H)),
        b3row=c(np.asarray(b3).reshape(1, H)),
        b4row=c(np.asarray(b4).reshape(1, H)),
        onesrow=c(np.ones((1, NI))),
        identp=c(np.eye(H)),
        gamma_rep=c(np.tile(np.asarray(gamma)[None, :], (H, 1))),
        beta_rep=c(np.tile(np.asarray(beta)[None, :], (H, 1))),
    )

    in_maps = []
    for core in range(8):
        b, ih = core // 2, core % 2
        i0 = ih * NI
        m = dict(shared)
        m["adj"] = c(adj_dist[b, i0:i0 + NI])
        m["x_all"] = c(x[b])
        m["xi"] = c(x[b, i0:i0 + NI])
        m["maskf"] = c(mask_np[b].astype(np.float32))
        m["condrep"] = c(np.tile(cond_vec[b][:, None], (1, H)))
        in_maps.append(m)

    nc = _get_program()
    _cache["in_maps"] = in_maps
    res = run_bass_kernel_spmd(nc, in_maps, list(range(8)))

    out_full = np.empty((B, N, H), dtype=np.float32)
    for core in range(8):
        b, ih = core // 2, core % 2
        out_full[b, ih * NI:(ih + 1) * NI] = res.results[core]["out"]
    return out_full



# revision 2
# speedup vs baseline: 1.4902x; 1.4902x over previous
"""CGNN layer kernel for Trainium2 (8 NeuronCores, SPMD) — v2.

Sharding: core c owns batch b = c//2 and receiver-node half i0 = (c%2)*128.

Host-side prep (layout only):
  - j-axis compaction: per batch, gather the live sender columns (mask==1)
    and pad to a common NJC (multiple of 8). Padded columns are zero; the
    on-device korr correction (which removes silu(bias) pollution from
    zeroed columns) covers them via the shipped 0/1 maskf.
  - adj is pre-transposed to the PE-ready stack layout
    stk[(g r), q, j] = adj[i0 + 4q + g, j, r], masked, scaled 1/SD, fp8.
  - x^T masked/scaled/fp8 for the x_j term; xi^T fp32 for the ACb term.
  - W1 is split and packed into 4 DoubleRow lhsT variants
    L_g = [w1bT*SW ; Z_g] fp8 where Z_g has W1dT*SD at partition band g.

Device math (per core, b fixed):
  z[i] (h=128, j=NJC) = ONE fp8 DoubleRow matmul:
      ktile0: (W1b*SW)^T @ (x^T*mask/SW)   [K=128]
      ktile1: Z_g^T @ stack_q              [K=128, band-selected adj term]
  silu + per-receiver bias ACb[:,i] + sum_j: ONE ACT op (bias + accum_out).
  ACb = W1a x_i + W1c c + b1 (fp32 matmuls, setup).
  S -= npad_or_dead * silu(ACb); aggr = W2 S + b2*live; update MLP + LN
  epilogue identical in structure to v1.
"""

import numpy as np
import ml_dtypes
from contextlib import ExitStack

import concourse.bass as bass
import concourse.bacc as bacc
import concourse.mybir as mybir
import concourse.tile as tile
from concourse.bass_utils import run_bass_kernel_spmd

ml_bf16 = ml_dtypes.bfloat16
ml_f8 = ml_dtypes.float8_e4m3

B, N, H, R = 4, 256, 128, 32
NI = 128          # receivers per core
NQ = NI // 4      # receiver quads
FP = mybir.dt.float32
BF = mybir.dt.bfloat16
F8 = mybir.dt.float8e4
EPS = 1e-5
ALU = mybir.AluOpType
ACTF = mybir.ActivationFunctionType
DR = mybir.MatmulPerfMode.DoubleRow

SW = 8.0   # fp8 scale for the W1b / x^T k-tile
SD = 8.0   # fp8 scale for the W1d / adj k-tile

_cache = {}


def _build_program(NJC):
    nc = bacc.Bacc()

    # ---- per-core DRAM parameters ----
    adj_stk = nc.declare_dram_parameter("adj_stk", [H, NQ, NJC], F8,
                                        isOutput=False)
    xT8 = nc.declare_dram_parameter("xT8", [H, NJC], F8, isOutput=False)
    xiT = nc.declare_dram_parameter("xiT", [H, NI], FP, isOutput=False)
    maskf = nc.declare_dram_parameter("maskf", [NJC], FP, isOutput=False)
    condrep = nc.declare_dram_parameter("condrep", [2 * H, H], FP,
                                        isOutput=False)
    lhs8 = nc.declare_dram_parameter("lhs8", [H, 4, 2, H], F8, isOutput=False)
    w1aT = nc.declare_dram_parameter("w1aT", [H, H], FP, isOutput=False)
    w1cT = nc.declare_dram_parameter("w1cT", [2 * H, H], FP, isOutput=False)
    w2T = nc.declare_dram_parameter("w2T", [H, H], FP, isOutput=False)
    w3aT = nc.declare_dram_parameter("w3aT", [H, H], FP, isOutput=False)
    w3bT = nc.declare_dram_parameter("w3bT", [H, H], FP, isOutput=False)
    w4T = nc.declare_dram_parameter("w4T", [H, H], FP, isOutput=False)
    b1row = nc.declare_dram_parameter("b1row", [1, H], FP, isOutput=False)
    b2row = nc.declare_dram_parameter("b2row", [1, H], FP, isOutput=False)
    b3row = nc.declare_dram_parameter("b3row", [1, H], FP, isOutput=False)
    b4row = nc.declare_dram_parameter("b4row", [1, H], FP, isOutput=False)
    onesrow = nc.declare_dram_parameter("onesrow", [1, NI], FP,
                                        isOutput=False)
    identp = nc.declare_dram_parameter("identp", [H, H], FP, isOutput=False)
    gamma_rep = nc.declare_dram_parameter("gamma_rep", [H, H], FP,
                                          isOutput=False)
    beta_rep = nc.declare_dram_parameter("beta_rep", [H, H], FP,
                                         isOutput=False)
    out = nc.declare_dram_parameter("out", [NI, H], FP, isOutput=True)

    with ExitStack() as ctx:
        tc = ctx.enter_context(tile.TileContext(nc))
        const = ctx.enter_context(tc.tile_pool(name="const", bufs=1))
        persist = ctx.enter_context(tc.tile_pool(name="persist", bufs=1))
        work = ctx.enter_context(tc.tile_pool(name="work", bufs=2))
        pep = ctx.enter_context(tc.tile_pool(name="pep", bufs=2,
                                             space="PSUM"))
        pz = ctx.enter_context(tc.tile_pool(name="pz", bufs=6, space="PSUM"))

        def cload(ap, shape, tag, dt=FP):
            if not isinstance(ap, bass.AP):
                ap = ap[:]
            t = const.tile(shape, dt, tag=tag, name=tag)
            nc.sync.dma_start(out=t, in_=ap)
            return t

        ident_sb = cload(identp, [H, H], "ident")
        w1aT_sb = cload(w1aT, [H, H], "w1aT")
        w1cT_sb0 = cload(w1cT[0:H, :], [H, H], "w1cT0")
        w1cT_sb1 = cload(w1cT[H:2 * H, :], [H, H], "w1cT1")
        w2T_sb = cload(w2T, [H, H], "w2T")
        w3aT_sb = cload(w3aT, [H, H], "w3aT")
        w3bT_sb = cload(w3bT, [H, H], "w3bT")
        w4T_sb = cload(w4T, [H, H], "w4T")
        condrep_sb0 = cload(condrep[0:H, :], [H, H], "condrep0")
        condrep_sb1 = cload(condrep[H:2 * H, :], [H, H], "condrep1")
        b1r_sb = cload(b1row, [1, H], "b1r")
        b2r_sb = cload(b2row, [1, H], "b2r")
        b3r_sb = cload(b3row, [1, H], "b3r")
        b4r_sb = cload(b4row, [1, H], "b4r")
        ones_sb = cload(onesrow, [1, NI], "onesr")
        xiT_sb = cload(xiT, [H, NI], "xiT")
        gamma_sb = cload(gamma_rep, [H, H], "gamma_rep")
        beta_sb = cload(beta_rep, [H, H], "beta_rep")
        lhs8_sb = cload(lhs8, [H, 4, 2, H], "lhs8", dt=F8)

        # rhs "big" tile: slot 0 = x^T (masked, /SW, fp8); slots 1..NQ = adj
        # stacks. DMA'd directly from host-prepped DRAM.
        rhsbig = persist.tile([H, NQ + 1, NJC], F8, tag="rhsbig",
                              name="rhsbig")
        nc.sync.dma_start(out=rhsbig[:, 0], in_=xT8[:])
        # split stack load across queues for parallelism
        nc.sync.dma_start(out=rhsbig[:, 1:1 + NQ // 2],
                          in_=adj_stk[:, 0:NQ // 2])
        nc.gpsimd.dma_start(out=rhsbig[:, 1 + NQ // 2:1 + NQ],
                            in_=adj_stk[:, NQ // 2:NQ])

        # mask broadcast to all partitions: (128, NJC)
        maskrep = persist.tile([H, NJC], FP, tag="maskrep", name="maskrep")
        maskf_ap = maskf[:]
        mask_bcast = bass.AP(tensor=maskf_ap.tensor, offset=maskf_ap.offset,
                             ap=[[0, H]] + list(maskf_ap.ap))
        nc.gpsimd.dma_start(out=maskrep, in_=mask_bcast)

        # per-partition live-count and dead/pad-count of sender slots
        msum = persist.tile([H, 1], FP, tag="msum", name="msum")
        mrow_scr = persist.tile([H, NJC], FP, tag="mrow_scr", name="mrow_scr")
        nc.vector.tensor_scalar(mrow_scr, maskrep, 1.0, None,
                                ALU.mult, ALU.add, accum_out=msum)
        nm0col = persist.tile([H, 1], FP, tag="nm0col", name="nm0col")
        nc.vector.tensor_scalar(nm0col, msum, -1.0, float(NJC),
                                ALU.mult, ALU.add)
        msum_row = persist.tile([1, NI], FP, tag="msum_row", name="msum_row")
        nc.vector.tensor_scalar(msum_row, ones_sb, msum[0:1, 0:1], None,
                                ALU.mult)

        ACb = persist.tile([H, NI], FP, tag="ACb", name="ACb")
        siluAC = persist.tile([H, NI], FP, tag="siluAC", name="siluAC")
        korr = persist.tile([H, NI], FP, tag="korr", name="korr")
        S_raw = persist.tile([H, NI], FP, tag="S_raw", name="S_raw")

        # ACb = W1a x_i + W1c c + b1  -> (128 h, 128 i)
        pA = pep.tile([H, NI], FP, tag="ps", name="pA")
        nc.tensor.matmul(pA, lhsT=w1aT_sb, rhs=xiT_sb, start=True, stop=False)
        nc.tensor.matmul(pA, lhsT=w1cT_sb0, rhs=condrep_sb0,
                         start=False, stop=False)
        nc.tensor.matmul(pA, lhsT=w1cT_sb1, rhs=condrep_sb1,
                         start=False, stop=False)
        nc.tensor.matmul(pA, lhsT=b1r_sb, rhs=ones_sb,
                         start=False, stop=True)
        nc.scalar.activation(ACb, pA, ACTF.Copy)

        # korr[h,i] = (dead+pad count) * silu(ACb[h,i])
        nc.scalar.activation(siluAC, ACb, ACTF.Silu)
        nc.vector.tensor_scalar(korr, siluAC, nm0col, None, ALU.mult)

        # ---- main loop: one DoubleRow matmul + one ACT per receiver ----
        for q in range(NQ):
            rhs_q = rhsbig[:, 0:q + 2:q + 1]   # slots {0, q+1}
            for g in range(4):
                li = 4 * q + g
                zt = pz.tile([H, NJC], FP, tag="zt", name="zt")
                nc.tensor.matmul(zt, lhsT=lhs8_sb[:, g], rhs=rhs_q,
                                 start=True, stop=True, perf_mode=DR)
                nc.scalar.activation(zt, zt, ACTF.Silu,
                                     bias=ACb[:, li:li + 1],
                                     accum_out=S_raw[:, li:li + 1])

        # ---- epilogue ----
        S_true = persist.tile([H, NI], FP, tag="S_true", name="S_true")
        nc.vector.scalar_tensor_tensor(out=S_true, in0=S_raw, scalar=0.0,
                                       in1=korr, op0=ALU.add,
                                       op1=ALU.subtract)
        # aggr = W2 s + b2 * live_count
        pa = pep.tile([H, NI], FP, tag="ps", name="pa")
        nc.tensor.matmul(pa, lhsT=w2T_sb, rhs=S_true, start=True, stop=False)
        nc.tensor.matmul(pa, lhsT=b2r_sb, rhs=msum_row, start=False,
                         stop=True)
        aggrT = work.tile([H, NI], FP, tag="aggrT", name="aggrT")
        nc.scalar.activation(aggrT, pa, ACTF.Copy)

        pu = pep.tile([H, NI], FP, tag="ps", name="pu")
        nc.tensor.matmul(pu, lhsT=w3aT_sb, rhs=xiT_sb, start=True, stop=False)
        nc.tensor.matmul(pu, lhsT=w3bT_sb, rhs=aggrT, start=False, stop=False)
        nc.tensor.matmul(pu, lhsT=b3r_sb, rhs=ones_sb, start=False,
                         stop=True)
        u_sb = work.tile([H, NI], FP, tag="u_sb", name="u_sb")
        nc.scalar.activation(u_sb, pu, ACTF.Silu)

        pupd = pep.tile([H, NI], FP, tag="ps", name="pupd")
        nc.tensor.matmul(pupd, lhsT=w4T_sb, rhs=u_sb, start=True, stop=False)
        nc.tensor.matmul(pupd, lhsT=b4r_sb, rhs=ones_sb, start=False,
                         stop=True)
        updT = work.tile([H, NI], FP, tag="updT", name="updT")
        nc.scalar.activation(updT, pupd, ACTF.Copy)

        py = pep.tile([NI, H], FP, tag="ps", name="py")
        nc.tensor.transpose(py, updT, ident_sb)

        # y = x + upd; LayerNorm over h (free dim)
        xi_row = persist.tile([NI, H], FP, tag="xi_row", name="xi_row")
        pxir = pep.tile([NI, H], FP, tag="ps", name="pxir")
        nc.tensor.transpose(pxir, xiT_sb, ident_sb)
        nc.vector.tensor_copy(xi_row, pxir)

        y_sb = work.tile([NI, H], FP, tag="y_sb", name="y_sb")
        rowsum = work.tile([NI, 1], FP, tag="rowsum", name="rowsum")
        nc.vector.scalar_tensor_tensor(out=y_sb, in0=py, scalar=0.0,
                                       in1=xi_row, op0=ALU.add, op1=ALU.add,
                                       accum_out=rowsum)
        negmu = work.tile([NI, 1], FP, tag="negmu", name="negmu")
        nc.vector.tensor_scalar(negmu, rowsum, -1.0 / H, None, ALU.mult)

        ysq = work.tile([NI, H], FP, tag="ysq", name="ysq")
        sumsq = work.tile([NI, 1], FP, tag="sumsq", name="sumsq")
        nc.vector.scalar_tensor_tensor(out=ysq, in0=y_sb, scalar=0.0,
                                       in1=y_sb, op0=ALU.add, op1=ALU.mult,
                                       accum_out=sumsq)
        ex2 = work.tile([NI, 1], FP, tag="ex2", name="ex2")
        nc.vector.tensor_scalar(ex2, sumsq, 1.0 / H, float(EPS),
                                ALU.mult, ALU.add)
        musq = work.tile([NI, 1], FP, tag="musq", name="musq")
        nc.vector.scalar_tensor_tensor(out=musq, in0=negmu, scalar=0.0,
                                       in1=negmu, op0=ALU.add, op1=ALU.mult)
        vare = work.tile([NI, 1], FP, tag="vare", name="vare")
        nc.vector.scalar_tensor_tensor(out=vare, in0=ex2, scalar=0.0,
                                       in1=musq, op0=ALU.add,
                                       op1=ALU.subtract)
        sd = work.tile([NI, 1], FP, tag="sd", name="sd")
        nc.scalar.activation(sd, vare, ACTF.Sqrt)
        rstd = work.tile([NI, 1], FP, tag="rstd", name="rstd")
        nc.vector.reciprocal(rstd, sd)

        yn = work.tile([NI, H], FP, tag="yn", name="yn")
        nc.vector.tensor_scalar(yn, y_sb, negmu, rstd, ALU.add, ALU.mult)
        yg = work.tile([NI, H], FP, tag="yg", name="yg")
        nc.vector.scalar_tensor_tensor(out=yg, in0=yn, scalar=0.0,
                                       in1=gamma_sb, op0=ALU.add,
                                       op1=ALU.mult)
        yfin = work.tile([NI, H], FP, tag="yfin", name="yfin")
        nc.vector.scalar_tensor_tensor(out=yfin, in0=yg, scalar=0.0,
                                       in1=beta_sb, op0=ALU.add,
                                       op1=ALU.add)
        nc.sync.dma_start(out=out[:], in_=yfin)

    nc.finalize()
    return nc


def _get_program(NJC):
    key = ("nc", NJC)
    if key not in _cache:
        _cache[key] = _build_program(NJC)
    return _cache[key]


def kernel(x, adj_dist, mask, cond_vec, W1, b1, W2, b2, W3, b3, W4, b4,
           gamma, beta):
    x = np.asarray(x, dtype=np.float32)
    adj_dist = np.asarray(adj_dist, dtype=np.float32)
    mask_np = np.asarray(mask)
    cond_vec = np.asarray(cond_vec, dtype=np.float32)
    W1 = np.asarray(W1, dtype=np.float32)
    W2 = np.asarray(W2, dtype=np.float32)
    W3 = np.asarray(W3, dtype=np.float32)
    W4 = np.asarray(W4, dtype=np.float32)

    def c(a):
        return np.ascontiguousarray(a, dtype=np.float32)

    # j-axis compaction: live sender indices per batch, padded to common NJC
    live_idx = [np.where(mask_np[b] != 0)[0] for b in range(B)]
    NJC = max(8, -(-max(len(ix) for ix in live_idx) // 8) * 8)

    # DoubleRow lhsT: [H(k), 4(g), 2(ktile), H(m)] fp8
    w1bT = W1[:, H:2 * H].T               # (k, h)
    w1dT = W1[:, 2 * H:2 * H + R].T       # (r, h)
    lhs8 = np.zeros((H, 4, 2, H), dtype=np.float32)
    lhs8[:, :, 0, :] = (w1bT * SW)[:, None, :]
    for g in range(4):
        lhs8[32 * g:32 * g + 32, g, 1, :] = w1dT * SD
    lhs8 = lhs8.astype(ml_f8)

    shared = dict(
        lhs8=lhs8,
        w1aT=c(W1[:, 0:H].T),
        w1cT=c(W1[:, 2 * H + R:].T),
        w2T=c(W2.T), w3aT=c(W3[:, 0:H].T), w3bT=c(W3[:, H:2 * H].T),
        w4T=c(W4.T),
        b1row=c(np.asarray(b1).reshape(1, H)),
        b2row=c(np.asarray(b2).reshape(1, H)),
        b3row=c(np.asarray(b3).reshape(1, H)),
        b4row=c(np.asarray(b4).reshape(1, H)),
        onesrow=c(np.ones((1, NI))),
        identp=c(np.eye(H)),
        gamma_rep=c(np.tile(np.asarray(gamma)[None, :], (H, 1))),
        beta_rep=c(np.tile(np.asarray(beta)[None, :], (H, 1))),
    )

    in_maps = []
    for core in range(8):
        b, ih = core // 2, core % 2
        i0 = ih * NI
        ix = live_idx[b]
        nlive = len(ix)

        # gathered x^T, masked (pad cols zero), scaled, fp8
        xg = np.zeros((H, NJC), dtype=np.float32)
        xg[:, :nlive] = x[b][ix].T / SW
        # adj stacks: [(g r), q, j] = adj[i0+4q+g, j_live, r] / SD
        ag = adj_dist[b, i0:i0 + NI][:, ix, :]          # (128, nlive, R)
        stk = np.zeros((H, NQ, NJC), dtype=np.float32)
        # i = 4q + g ; partition p = 32g + r
        a4 = ag.reshape(NQ, 4, nlive, R)                # (q, g, j, r)
        stk[:, :, :nlive] = (a4.transpose(1, 3, 0, 2)   # (g, r, q, j)
                             .reshape(H, NQ, nlive)) / SD
        mf = np.zeros((NJC,), dtype=np.float32)
        mf[:nlive] = 1.0

        m = dict(shared)
        m["adj_stk"] = stk.astype(ml_f8)
        m["xT8"] = xg.astype(ml_f8)
        m["xiT"] = c(x[b, i0:i0 + NI].T)
        m["maskf"] = mf
        m["condrep"] = c(np.tile(cond_vec[b][:, None], (1, H)))
        in_maps.append(m)

    nc = _get_program(NJC)
    _cache["in_maps"] = in_maps
    _cache["last_njc"] = NJC
    res = run_bass_kernel_spmd(nc, in_maps, list(range(8)))

    out_full = np.empty((B, N, H), dtype=np.float32)
    for core in range(8):
        b, ih = core // 2, core % 2
        out_full[b, ih * NI:(ih + 1) * NI] = res.results[core]["out"]
    return out_full


# revision 3
# speedup vs baseline: 1.9772x; 1.3268x over previous
"""CGNN layer kernel for Trainium2 (8 NeuronCores, SPMD) — v2.

Sharding: core c owns batch b = c//2 and receiver-node half i0 = (c%2)*128.

Host-side prep (layout only):
  - j-axis compaction: per batch, gather the live sender columns (mask==1)
    and pad to a common NJC (multiple of 8). Padded columns are zero; the
    on-device korr correction (which removes silu(bias) pollution from
    zeroed columns) covers them via the shipped 0/1 maskf.
  - adj is pre-transposed to the PE-ready stack layout
    stk[(g r), q, j] = adj[i0 + 4q + g, j, r], masked, scaled 1/SD, fp8.
  - x^T masked/scaled/fp8 for the x_j term; xi^T fp32 for the ACb term.
  - W1 is split and packed into 4 DoubleRow lhsT variants
    L_g = [w1bT*SW ; Z_g] fp8 where Z_g has W1dT*SD at partition band g.

Device math (per core, b fixed):
  z[i] (h=128, j=NJC) = ONE fp8 DoubleRow matmul:
      ktile0: (W1b*SW)^T @ (x^T*mask/SW)   [K=128]
      ktile1: Z_g^T @ stack_q              [K=128, band-selected adj term]
  silu + per-receiver bias ACb[:,i] + sum_j: ONE ACT op (bias + accum_out).
  ACb = W1a x_i + W1c c + b1 (fp32 matmuls, setup).
  S -= npad_or_dead * silu(ACb); aggr = W2 S + b2*live; update MLP + LN
  epilogue identical in structure to v1.
"""

import numpy as np
import ml_dtypes
from contextlib import ExitStack

import concourse.bass as bass
import concourse.bacc as bacc
import concourse.mybir as mybir
import concourse.tile as tile
from concourse.bass_utils import run_bass_kernel_spmd

ml_bf16 = ml_dtypes.bfloat16
ml_f8 = ml_dtypes.float8_e4m3

B, N, H, R = 4, 256, 128, 32
NI = 128          # receivers per core
NQ = NI // 4      # receiver quads
FP = mybir.dt.float32
BF = mybir.dt.bfloat16
F8 = mybir.dt.float8e4
EPS = 1e-5
ALU = mybir.AluOpType
ACTF = mybir.ActivationFunctionType
DR = mybir.MatmulPerfMode.DoubleRow

SW = 8.0   # fp8 scale for the W1b / x^T k-tile
SD = 8.0   # fp8 scale for the W1d / adj k-tile

_cache = {}


def _build_program(NJC):
    nc = bacc.Bacc()

    # ---- per-core DRAM parameters ----
    adj_stk = nc.declare_dram_parameter("adj_stk", [H, NQ, NJC], F8,
                                        isOutput=False)
    xT8 = nc.declare_dram_parameter("xT8", [H, NJC], F8, isOutput=False)
    xiT = nc.declare_dram_parameter("xiT", [H, NI], FP, isOutput=False)
    maskf = nc.declare_dram_parameter("maskf", [NJC], FP, isOutput=False)
    condrep = nc.declare_dram_parameter("condrep", [2 * H, H], FP,
                                        isOutput=False)
    lhs8 = nc.declare_dram_parameter("lhs8", [H, 4, 2, H], F8, isOutput=False)
    w1aT = nc.declare_dram_parameter("w1aT", [H, H], FP, isOutput=False)
    w1cT = nc.declare_dram_parameter("w1cT", [2 * H, H], FP, isOutput=False)
    w2T = nc.declare_dram_parameter("w2T", [H, H], FP, isOutput=False)
    w3aT = nc.declare_dram_parameter("w3aT", [H, H], FP, isOutput=False)
    w3bT = nc.declare_dram_parameter("w3bT", [H, H], FP, isOutput=False)
    w4T = nc.declare_dram_parameter("w4T", [H, H], FP, isOutput=False)
    b1row = nc.declare_dram_parameter("b1row", [1, H], FP, isOutput=False)
    b2row = nc.declare_dram_parameter("b2row", [1, H], FP, isOutput=False)
    b3row = nc.declare_dram_parameter("b3row", [1, H], FP, isOutput=False)
    b4row = nc.declare_dram_parameter("b4row", [1, H], FP, isOutput=False)
    onesrow = nc.declare_dram_parameter("onesrow", [1, NI], FP,
                                        isOutput=False)
    identp = nc.declare_dram_parameter("identp", [H, H], FP, isOutput=False)
    gamma_rep = nc.declare_dram_parameter("gamma_rep", [H, H], FP,
                                          isOutput=False)
    beta_rep = nc.declare_dram_parameter("beta_rep", [H, H], FP,
                                         isOutput=False)
    out = nc.declare_dram_parameter("out", [NI, H], FP, isOutput=True)

    with ExitStack() as ctx:
        tc = ctx.enter_context(tile.TileContext(nc))
        const = ctx.enter_context(tc.tile_pool(name="const", bufs=1))
        persist = ctx.enter_context(tc.tile_pool(name="persist", bufs=1))
        work = ctx.enter_context(tc.tile_pool(name="work", bufs=2))
        pep = ctx.enter_context(tc.tile_pool(name="pep", bufs=2,
                                             space="PSUM"))
        pz = ctx.enter_context(tc.tile_pool(name="pz", bufs=6, space="PSUM"))

        def cload(ap, shape, tag, dt=FP):
            if not isinstance(ap, bass.AP):
                ap = ap[:]
            t = const.tile(shape, dt, tag=tag, name=tag)
            nc.sync.dma_start(out=t, in_=ap)
            return t

        ident_sb = cload(identp, [H, H], "ident")
        w1aT_sb = cload(w1aT, [H, H], "w1aT")
        w1cT_sb0 = cload(w1cT[0:H, :], [H, H], "w1cT0")
        w1cT_sb1 = cload(w1cT[H:2 * H, :], [H, H], "w1cT1")
        w2T_sb = cload(w2T, [H, H], "w2T")
        w3aT_sb = cload(w3aT, [H, H], "w3aT")
        w3bT_sb = cload(w3bT, [H, H], "w3bT")
        w4T_sb = cload(w4T, [H, H], "w4T")
        condrep_sb0 = cload(condrep[0:H, :], [H, H], "condrep0")
        condrep_sb1 = cload(condrep[H:2 * H, :], [H, H], "condrep1")
        b1r_sb = cload(b1row, [1, H], "b1r")
        b2r_sb = cload(b2row, [1, H], "b2r")
        b3r_sb = cload(b3row, [1, H], "b3r")
        b4r_sb = cload(b4row, [1, H], "b4r")
        ones_sb = cload(onesrow, [1, NI], "onesr")
        xiT_sb = cload(xiT, [H, NI], "xiT")
        gamma_sb = cload(gamma_rep, [H, H], "gamma_rep")
        beta_sb = cload(beta_rep, [H, H], "beta_rep")
        lhs8_sb = cload(lhs8, [H, 4, 2, H], "lhs8", dt=F8)

        # rhs "big" tile: slot 0 = x^T (masked, /SW, fp8); slots 1..NQ = adj
        # stacks. DMA'd directly from host-prepped DRAM.
        rhsbig = persist.tile([H, NQ + 1, NJC], F8, tag="rhsbig",
                              name="rhsbig")
        nc.sync.dma_start(out=rhsbig[:, 0], in_=xT8[:])
        # split stack load across queues for parallelism
        nc.sync.dma_start(out=rhsbig[:, 1:1 + NQ // 2],
                          in_=adj_stk[:, 0:NQ // 2])
        nc.gpsimd.dma_start(out=rhsbig[:, 1 + NQ // 2:1 + NQ],
                            in_=adj_stk[:, NQ // 2:NQ])

        # mask broadcast to all partitions: (128, NJC)
        maskrep = persist.tile([H, NJC], FP, tag="maskrep", name="maskrep")
        maskf_ap = maskf[:]
        mask_bcast = bass.AP(tensor=maskf_ap.tensor, offset=maskf_ap.offset,
                             ap=[[0, H]] + list(maskf_ap.ap))
        nc.gpsimd.dma_start(out=maskrep, in_=mask_bcast)

        # per-partition live-count and dead/pad-count of sender slots
        msum = persist.tile([H, 1], FP, tag="msum", name="msum")
        mrow_scr = persist.tile([H, NJC], FP, tag="mrow_scr", name="mrow_scr")
        nc.vector.tensor_scalar(mrow_scr, maskrep, 1.0, None,
                                ALU.mult, ALU.add, accum_out=msum)
        nm0col = persist.tile([H, 1], FP, tag="nm0col", name="nm0col")
        nc.vector.tensor_scalar(nm0col, msum, -1.0, float(NJC),
                                ALU.mult, ALU.add)
        msum_row = persist.tile([1, NI], FP, tag="msum_row", name="msum_row")
        nc.vector.tensor_scalar(msum_row, ones_sb, msum[0:1, 0:1], None,
                                ALU.mult)

        ACb = persist.tile([H, NI], FP, tag="ACb", name="ACb")
        siluAC = persist.tile([H, NI], FP, tag="siluAC", name="siluAC")
        korr = persist.tile([H, NI], FP, tag="korr", name="korr")
        S_raw = persist.tile([H, NI], FP, tag="S_raw", name="S_raw")

        # ACb = W1a x_i + W1c c + b1  -> (128 h, 128 i)
        pA = pep.tile([H, NI], FP, tag="ps", name="pA")
        nc.tensor.matmul(pA, lhsT=w1aT_sb, rhs=xiT_sb, start=True, stop=False)
        nc.tensor.matmul(pA, lhsT=w1cT_sb0, rhs=condrep_sb0,
                         start=False, stop=False)
        nc.tensor.matmul(pA, lhsT=w1cT_sb1, rhs=condrep_sb1,
                         start=False, stop=False)
        nc.tensor.matmul(pA, lhsT=b1r_sb, rhs=ones_sb,
                         start=False, stop=True)
        nc.scalar.activation(ACb, pA, ACTF.Copy)

        # korr[h,i] = (dead+pad count) * silu(ACb[h,i])
        nc.scalar.activation(siluAC, ACb, ACTF.Silu)
        nc.vector.tensor_scalar(korr, siluAC, nm0col, None, ALU.mult)

        # ---- main loop: one DoubleRow matmul + one ACT per receiver;
        # one segmented DVE reduce (sum over j, 4 receivers) per quad ----
        scr = ctx.enter_context(tc.tile_pool(name="scr", bufs=3))
        for q in range(NQ):
            rhs_q = rhsbig[:, 0:q + 2:q + 1]   # slots {0, q+1}
            sinkq = scr.tile([H, 4, NJC], BF, tag="sinkq", name="sinkq")
            for g in range(4):
                li = 4 * q + g
                zt = pz.tile([H, NJC], FP, tag="zt", name="zt")
                nc.tensor.matmul(zt, lhsT=lhs8_sb[:, g], rhs=rhs_q,
                                 start=True, stop=True, perf_mode=DR)
                nc.scalar.activation(sinkq[:, g], zt, ACTF.Silu,
                                     bias=ACb[:, li:li + 1])
            nc.vector.tensor_reduce(S_raw[:, 4 * q:4 * q + 4], sinkq,
                                    mybir.AxisListType.X, ALU.add)

        # ---- epilogue ----
        S_true = persist.tile([H, NI], FP, tag="S_true", name="S_true")
        nc.vector.scalar_tensor_tensor(out=S_true, in0=S_raw, scalar=0.0,
                                       in1=korr, op0=ALU.add,
                                       op1=ALU.subtract)
        # aggr = W2 s + b2 * live_count
        pa = pep.tile([H, NI], FP, tag="ps", name="pa")
        nc.tensor.matmul(pa, lhsT=w2T_sb, rhs=S_true, start=True, stop=False)
        nc.tensor.matmul(pa, lhsT=b2r_sb, rhs=msum_row, start=False,
                         stop=True)
        aggrT = work.tile([H, NI], FP, tag="aggrT", name="aggrT")
        nc.scalar.activation(aggrT, pa, ACTF.Copy)

        pu = pep.tile([H, NI], FP, tag="ps", name="pu")
        nc.tensor.matmul(pu, lhsT=w3aT_sb, rhs=xiT_sb, start=True, stop=False)
        nc.tensor.matmul(pu, lhsT=w3bT_sb, rhs=aggrT, start=False, stop=False)
        nc.tensor.matmul(pu, lhsT=b3r_sb, rhs=ones_sb, start=False,
                         stop=True)
        u_sb = work.tile([H, NI], FP, tag="u_sb", name="u_sb")
        nc.scalar.activation(u_sb, pu, ACTF.Silu)

        pupd = pep.tile([H, NI], FP, tag="ps", name="pupd")
        nc.tensor.matmul(pupd, lhsT=w4T_sb, rhs=u_sb, start=True, stop=False)
        nc.tensor.matmul(pupd, lhsT=b4r_sb, rhs=ones_sb, start=False,
                         stop=True)
        updT = work.tile([H, NI], FP, tag="updT", name="updT")
        nc.scalar.activation(updT, pupd, ACTF.Copy)

        py = pep.tile([NI, H], FP, tag="ps", name="py")
        nc.tensor.transpose(py, updT, ident_sb)

        # y = x + upd; LayerNorm over h (free dim)
        xi_row = persist.tile([NI, H], FP, tag="xi_row", name="xi_row")
        pxir = pep.tile([NI, H], FP, tag="ps", name="pxir")
        nc.tensor.transpose(pxir, xiT_sb, ident_sb)
        nc.vector.tensor_copy(xi_row, pxir)

        y_sb = work.tile([NI, H], FP, tag="y_sb", name="y_sb")
        rowsum = work.tile([NI, 1], FP, tag="rowsum", name="rowsum")
        nc.vector.scalar_tensor_tensor(out=y_sb, in0=py, scalar=0.0,
                                       in1=xi_row, op0=ALU.add, op1=ALU.add,
                                       accum_out=rowsum)
        negmu = work.tile([NI, 1], FP, tag="negmu", name="negmu")
        nc.vector.tensor_scalar(negmu, rowsum, -1.0 / H, None, ALU.mult)

        ysq = work.tile([NI, H], FP, tag="ysq", name="ysq")
        sumsq = work.tile([NI, 1], FP, tag="sumsq", name="sumsq")
        nc.vector.scalar_tensor_tensor(out=ysq, in0=y_sb, scalar=0.0,
                                       in1=y_sb, op0=ALU.add, op1=ALU.mult,
                                       accum_out=sumsq)
        ex2 = work.tile([NI, 1], FP, tag="ex2", name="ex2")
        nc.vector.tensor_scalar(ex2, sumsq, 1.0 / H, float(EPS),
                                ALU.mult, ALU.add)
        musq = work.tile([NI, 1], FP, tag="musq", name="musq")
        nc.vector.scalar_tensor_tensor(out=musq, in0=negmu, scalar=0.0,
                                       in1=negmu, op0=ALU.add, op1=ALU.mult)
        vare = work.tile([NI, 1], FP, tag="vare", name="vare")
        nc.vector.scalar_tensor_tensor(out=vare, in0=ex2, scalar=0.0,
                                       in1=musq, op0=ALU.add,
                                       op1=ALU.subtract)
        sd = work.tile([NI, 1], FP, tag="sd", name="sd")
        nc.scalar.activation(sd, vare, ACTF.Sqrt)
        rstd = work.tile([NI, 1], FP, tag="rstd", name="rstd")
        nc.vector.reciprocal(rstd, sd)

        yn = work.tile([NI, H], FP, tag="yn", name="yn")
        nc.vector.tensor_scalar(yn, y_sb, negmu, rstd, ALU.add, ALU.mult)
        yg = work.tile([NI, H], FP, tag="yg", name="yg")
        nc.vector.scalar_tensor_tensor(out=yg, in0=yn, scalar=0.0,
                                       in1=gamma_sb, op0=ALU.add,
                                       op1=ALU.mult)
        yfin = work.tile([NI, H], FP, tag="yfin", name="yfin")
        nc.vector.scalar_tensor_tensor(out=yfin, in0=yg, scalar=0.0,
                                       in1=beta_sb, op0=ALU.add,
                                       op1=ALU.add)
        nc.sync.dma_start(out=out[:], in_=yfin)

    nc.finalize()
    return nc


def _get_program(NJC):
    key = ("nc", NJC)
    if key not in _cache:
        _cache[key] = _build_program(NJC)
    return _cache[key]


def kernel(x, adj_dist, mask, cond_vec, W1, b1, W2, b2, W3, b3, W4, b4,
           gamma, beta):
    x = np.asarray(x, dtype=np.float32)
    adj_dist = np.asarray(adj_dist, dtype=np.float32)
    mask_np = np.asarray(mask)
    cond_vec = np.asarray(cond_vec, dtype=np.float32)
    W1 = np.asarray(W1, dtype=np.float32)
    W2 = np.asarray(W2, dtype=np.float32)
    W3 = np.asarray(W3, dtype=np.float32)
    W4 = np.asarray(W4, dtype=np.float32)

    def c(a):
        return np.ascontiguousarray(a, dtype=np.float32)

    # j-axis compaction: live sender indices per batch, padded to common NJC
    live_idx = [np.where(mask_np[b] != 0)[0] for b in range(B)]
    NJC = max(8, -(-max(len(ix) for ix in live_idx) // 8) * 8)

    # DoubleRow lhsT: [H(k), 4(g), 2(ktile), H(m)] fp8
    w1bT = W1[:, H:2 * H].T               # (k, h)
    w1dT = W1[:, 2 * H:2 * H + R].T       # (r, h)
    lhs8 = np.zeros((H, 4, 2, H), dtype=np.float32)
    lhs8[:, :, 0, :] = (w1bT * SW)[:, None, :]
    for g in range(4):
        lhs8[32 * g:32 * g + 32, g, 1, :] = w1dT * SD
    lhs8 = lhs8.astype(ml_f8)

    shared = dict(
        lhs8=lhs8,
        w1aT=c(W1[:, 0:H].T),
        w1cT=c(W1[:, 2 * H + R:].T),
        w2T=c(W2.T), w3aT=c(W3[:, 0:H].T), w3bT=c(W3[:, H:2 * H].T),
        w4T=c(W4.T),
        b1row=c(np.asarray(b1).reshape(1, H)),
        b2row=c(np.asarray(b2).reshape(1, H)),
        b3row=c(np.asarray(b3).reshape(1, H)),
        b4row=c(np.asarray(b4).reshape(1, H)),
        onesrow=c(np.ones((1, NI))),
        identp=c(np.eye(H)),
        gamma_rep=c(np.tile(np.asarray(gamma)[None, :], (H, 1))),
        beta_rep=c(np.tile(np.asarray(beta)[None, :], (H, 1))),
    )

    in_maps = []
    for core in range(8):
        b, ih = core // 2, core % 2
        i0 = ih * NI
        ix = live_idx[b]
        nlive = len(ix)

        # gathered x^T, masked (pad cols zero), scaled, fp8
        xg = np.zeros((H, NJC), dtype=np.float32)
        xg[:, :nlive] = x[b][ix].T / SW
        # adj stacks: [(g r), q, j] = adj[i0+4q+g, j_live, r] / SD
        ag = adj_dist[b, i0:i0 + NI][:, ix, :]          # (128, nlive, R)
        stk = np.zeros((H, NQ, NJC), dtype=np.float32)
        # i = 4q + g ; partition p = 32g + r
        a4 = ag.reshape(NQ, 4, nlive, R)                # (q, g, j, r)
        stk[:, :, :nlive] = (a4.transpose(1, 3, 0, 2)   # (g, r, q, j)
                             .reshape(H, NQ, nlive)) / SD
        mf = np.zeros((NJC,), dtype=np.float32)
        mf[:nlive] = 1.0

        m = dict(shared)
        m["adj_stk"] = stk.astype(ml_f8)
        m["xT8"] = xg.astype(ml_f8)
        m["xiT"] = c(x[b, i0:i0 + NI].T)
        m["maskf"] = mf
        m["condrep"] = c(np.tile(cond_vec[b][:, None], (1, H)))
        in_maps.append(m)

    nc = _get_program(NJC)
    _cache["in_maps"] = in_maps
    _cache["last_njc"] = NJC
    res = run_bass_kernel_spmd(nc, in_maps, list(range(8)))

    out_full = np.empty((B, N, H), dtype=np.float32)
    for core in range(8):
        b, ih = core // 2, core % 2
        out_full[b, ih * NI:(ih + 1) * NI] = res.results[core]["out"]
    return out_full


# revision 8
# speedup vs baseline: 2.3149x; 1.1708x over previous
"""CGNN layer kernel for Trainium2 (8 NeuronCores, SPMD) — v4.

Sharding: core c owns batch b = c//2 and receiver-node half i0 = (c%2)*128.

Host-side prep (layout only):
  - j-axis compaction: per batch, gather the live sender columns (mask==1)
    and pad to a common NJC (multiple of 8). Padded columns are zero; the
    on-device korr correction (which removes silu(bias) pollution from
    zeroed columns) covers them via the shipped 0/1 maskf.
  - adj is pre-transposed to the PE-ready stack layout
    stk[(g r), q, j] = adj[i0 + 4q + g, j, r], masked, scaled 1/SD, fp8.
  - x^T masked/scaled/fp8 for the x_j term; xi^T fp32 for the ACb term.
  - W1 is split and packed into 4 DoubleRow lhsT variants
    L_g = [w1bT*SW ; Z_g] fp8 where Z_g has W1dT*SD at partition band g.
  - all small fp32 [H,H] consts ride in ONE packed DRAM param (one DMA);
    bias rows in another.

Device math (per core, b fixed):
  z[i] (h=128, j=NJC) = ONE fp8 DoubleRow matmul:
      ktile0: (W1b*SW)^T @ (x^T*mask/SW)   [K=128]
      ktile1: Z_g^T @ stack_q              [K=128, band-selected adj term]
  silu + per-receiver bias ACb[:,i]: ONE ACT op -> bf16 sink slice;
  per-quad segmented DVE tensor_reduce sums 4 receivers over j at once.
  ACb = W1a x_i + W1c c + b1 (fp32 matmuls, setup).
  S -= npad_or_dead * silu(ACb); aggr = W2 S + b2*live; update MLP (bf16
  matmuls) + LayerNorm epilogue.
"""

import numpy as np
import ml_dtypes
from contextlib import ExitStack

import concourse.bass as bass
import concourse.bacc as bacc
import concourse.mybir as mybir
import concourse.tile as tile
from concourse.bass_utils import run_bass_kernel_spmd

ml_bf16 = ml_dtypes.bfloat16
ml_f8 = ml_dtypes.float8_e4m3

B, N, H, R = 4, 256, 128, 32
NI = 128          # receivers per core
NQ = NI // 4      # receiver quads
FP = mybir.dt.float32
BF = mybir.dt.bfloat16
F8 = mybir.dt.float8e4
EPS = 1e-5
ALU = mybir.AluOpType
ACTF = mybir.ActivationFunctionType
DR = mybir.MatmulPerfMode.DoubleRow

SW = 8.0   # fp8 scale for the W1b / x^T k-tile
SD = 8.0   # fp8 scale for the W1d / adj k-tile

# packed fp32 [H,H] const slots
PK = ["w1aT", "w1cT0", "w1cT1", "condrep0", "condrep1", "identp",
      "gamma_rep", "beta_rep"]
# packed bf16 [H,H] const slots (epilogue matmul weights)
PKB = ["w2T", "w3aT", "w3bT", "w4T"]

_cache = {}


def _build_program(NJC):
    nc = bacc.Bacc()

    # ---- per-core DRAM parameters ----
    adj_stk = nc.declare_dram_parameter("adj_stk", [H, NQ, NJC], F8,
                                        isOutput=False)
    xT8 = nc.declare_dram_parameter("xT8", [H, NJC], F8, isOutput=False)
    xiT = nc.declare_dram_parameter("xiT", [H, NI], FP, isOutput=False)
    maskf = nc.declare_dram_parameter("maskf", [NJC], FP, isOutput=False)
    lhs8 = nc.declare_dram_parameter("lhs8", [H, 4, 2, H], F8, isOutput=False)
    packf = nc.declare_dram_parameter("packf", [H, len(PK), H], FP,
                                      isOutput=False)
    packb = nc.declare_dram_parameter("packb", [H, len(PKB), H], BF,
                                      isOutput=False)
    rows = nc.declare_dram_parameter("rows", [1, 6, H], FP, isOutput=False)
    rowsb = nc.declare_dram_parameter("rowsb", [1, 5, H], BF, isOutput=False)
    out = nc.declare_dram_parameter("out", [NI, H], FP, isOutput=True)

    with ExitStack() as ctx:
        tc = ctx.enter_context(tile.TileContext(nc))
        const = ctx.enter_context(tc.tile_pool(name="const", bufs=1))
        persist = ctx.enter_context(tc.tile_pool(name="persist", bufs=1))
        work = ctx.enter_context(tc.tile_pool(name="work", bufs=2))
        scr = ctx.enter_context(tc.tile_pool(name="scr", bufs=3))
        pep = ctx.enter_context(tc.tile_pool(name="pep", bufs=2,
                                             space="PSUM"))
        pz = ctx.enter_context(tc.tile_pool(name="pz", bufs=6, space="PSUM"))

        # rhs "big" tile: slot 0 = x^T (masked, /SW, fp8); slots 1..NQ = adj
        # stacks, DMA'd directly from host-prepped DRAM. Issue these FIRST
        # (they gate the PE main loop), split across queues.
        rhsbig = persist.tile([H, NQ + 1, NJC], F8, tag="rhsbig",
                              name="rhsbig")
        # ACb-critical loads first (small, gate the ACT pipeline)
        xiT_sb = const.tile([H, NI], FP, tag="xiT", name="xiT")
        nc.sync.dma_start(out=xiT_sb, in_=xiT[:])
        packf_sb = const.tile([H, len(PK), H], FP, tag="packf", name="packf")
        nc.scalar.dma_start(out=packf_sb, in_=packf[:])
        rows_sb = const.tile([1, 6, H], FP, tag="rows", name="rows")
        nc.scalar.dma_start(out=rows_sb, in_=rows[:])
        lhs8_sb = const.tile([H, 4, 2, H], F8, tag="lhs8", name="lhs8")
        nc.gpsimd.dma_start(out=lhs8_sb, in_=lhs8[:])

        nc.sync.dma_start(out=rhsbig[:, 0], in_=xT8[:])
        CH = NQ // 4
        qeng = [nc.sync, nc.gpsimd, nc.sync, nc.gpsimd]
        for ci in range(4):
            qeng[ci].dma_start(
                out=rhsbig[:, 1 + ci * CH:1 + (ci + 1) * CH],
                in_=adj_stk[:, ci * CH:(ci + 1) * CH])

        packb_sb = const.tile([H, len(PKB), H], BF, tag="packb", name="packb")
        nc.sync.dma_start(out=packb_sb, in_=packb[:])
        rowsb_sb = const.tile([1, 5, H], BF, tag="rowsb", name="rowsb")
        nc.gpsimd.dma_start(out=rowsb_sb, in_=rowsb[:])

        pk = {name: packf_sb[:, i] for i, name in enumerate(PK)}
        pkb = {name: packb_sb[:, i] for i, name in enumerate(PKB)}
        b1r = rows_sb[0:1, 0]
        ones_r = rows_sb[0:1, 4]
        eps_r = rows_sb[0:1, 5]
        b2rb = rowsb_sb[0:1, 1]
        b3rb = rowsb_sb[0:1, 2]
        b4rb = rowsb_sb[0:1, 3]
        onesb_r = rowsb_sb[0:1, 4]

        # mask broadcast to all partitions: (128, NJC)
        maskrep = persist.tile([H, NJC], FP, tag="maskrep", name="maskrep")
        maskf_ap = maskf[:]
        mask_bcast = bass.AP(tensor=maskf_ap.tensor, offset=maskf_ap.offset,
                             ap=[[0, H]] + list(maskf_ap.ap))
        nc.gpsimd.dma_start(out=maskrep, in_=mask_bcast)

        # Preload the Silu ACT table early (hidden under DMA) with a dummy
        # op on the eps row (already-loaded const).
        dummy = work.tile([1, H], FP, tag="dummy", name="dummy")
        nc.scalar.activation(dummy, eps_r, ACTF.Silu)

        # per-partition live-count and dead/pad-count of sender slots
        msum = persist.tile([H, 1], FP, tag="msum", name="msum")
        mrow_scr = persist.tile([H, NJC], FP, tag="mrow_scr", name="mrow_scr")
        nc.vector.tensor_scalar(mrow_scr, maskrep, 1.0, None,
                                ALU.mult, ALU.add, accum_out=msum)
        nm0col = persist.tile([H, 1], FP, tag="nm0col", name="nm0col")
        nc.vector.tensor_scalar(nm0col, msum, -1.0, float(NJC),
                                ALU.mult, ALU.add)
        msum_rowb = persist.tile([1, NI], BF, tag="msum_rowb",
                                 name="msum_rowb")
        nc.vector.tensor_scalar(msum_rowb, ones_r, msum[0:1, 0:1], None,
                                ALU.mult)

        ACb = persist.tile([H, NI], FP, tag="ACb", name="ACb")
        siluAC = persist.tile([H, NI], FP, tag="siluAC", name="siluAC")
        korr = persist.tile([H, NI], FP, tag="korr", name="korr")
        S_raw = persist.tile([H, NI], FP, tag="S_raw", name="S_raw")

        # ACb = W1a x_i + W1c c + b1  -> (128 h, 128 i)
        pA = pep.tile([H, NI], FP, tag="ps", name="pA")
        nc.tensor.matmul(pA, lhsT=pk["w1aT"], rhs=xiT_sb,
                         start=True, stop=False)
        nc.tensor.matmul(pA, lhsT=pk["w1cT0"], rhs=pk["condrep0"],
                         start=False, stop=False)
        nc.tensor.matmul(pA, lhsT=pk["w1cT1"], rhs=pk["condrep1"],
                         start=False, stop=False)
        nc.tensor.matmul(pA, lhsT=b1r, rhs=ones_r,
                         start=False, stop=True)
        nc.vector.tensor_copy(ACb, pA)

        # korr[h,i] = (dead+pad count) * silu(ACb[h,i])
        nc.scalar.activation(siluAC, ACb, ACTF.Silu)
        nc.vector.tensor_scalar(korr, siluAC, nm0col, None, ALU.mult)

        # ---- main loop: one DoubleRow matmul + one ACT per receiver;
        # one segmented DVE reduce (sum over j, 4 receivers) per quad ----
        for q in range(NQ):
            rhs_q = rhsbig[:, 0:q + 2:q + 1]   # slots {0, q+1}
            sinkq = scr.tile([H, 4, NJC], BF, tag="sinkq", name="sinkq")
            for g in range(4):
                li = 4 * q + g
                zt = pz.tile([H, NJC], FP, tag="zt", name="zt")
                nc.tensor.matmul(zt, lhsT=lhs8_sb[:, g], rhs=rhs_q,
                                 start=True, stop=True, perf_mode=DR)
                nc.scalar.activation(sinkq[:, g], zt, ACTF.Silu,
                                     bias=ACb[:, li:li + 1])
            nc.vector.tensor_reduce(S_raw[:, 4 * q:4 * q + 4], sinkq,
                                    mybir.AxisListType.X, ALU.add)

        # ---- epilogue ----
        S_true = persist.tile([H, NI], BF, tag="S_true", name="S_true")
        nc.vector.scalar_tensor_tensor(out=S_true, in0=S_raw, scalar=0.0,
                                       in1=korr, op0=ALU.add,
                                       op1=ALU.subtract)
        # aggr = W2 s + b2 * live_count
        pa = pep.tile([H, NI], FP, tag="ps", name="pa")
        nc.tensor.matmul(pa, lhsT=pkb["w2T"], rhs=S_true,
                         start=True, stop=False)
        nc.tensor.matmul(pa, lhsT=b2rb, rhs=msum_rowb, start=False,
                         stop=True)
        aggrT = work.tile([H, NI], BF, tag="aggrT", name="aggrT")
        nc.vector.tensor_copy(aggrT, pa)
        xiTb = work.tile([H, NI], BF, tag="xiTb", name="xiTb")
        nc.gpsimd.tensor_copy(xiTb, xiT_sb)
        onesNIb = rowsb_sb[0:1, 4]   # bf16 ones row (NI == H)

        pu = pep.tile([H, NI], FP, tag="ps", name="pu")
        nc.tensor.matmul(pu, lhsT=pkb["w3aT"], rhs=xiTb,
                         start=True, stop=False)
        nc.tensor.matmul(pu, lhsT=pkb["w3bT"], rhs=aggrT,
                         start=False, stop=False)
        nc.tensor.matmul(pu, lhsT=b3rb, rhs=onesNIb, start=False,
                         stop=True)
        u_sb = work.tile([H, NI], BF, tag="u_sb", name="u_sb")
        nc.scalar.activation(u_sb, pu, ACTF.Silu)

        pupd = pep.tile([H, NI], FP, tag="ps", name="pupd")
        nc.tensor.matmul(pupd, lhsT=pkb["w4T"], rhs=u_sb,
                         start=True, stop=False)
        nc.tensor.matmul(pupd, lhsT=b4rb, rhs=onesNIb, start=False,
                         stop=True)
        updT = work.tile([H, NI], FP, tag="updT", name="updT")
        nc.vector.tensor_copy(updT, pupd)

        py = pep.tile([NI, H], FP, tag="ps", name="py")
        nc.tensor.transpose(py, updT, pk["identp"])

        # y = x + upd; LayerNorm over h (free dim)
        xi_row = persist.tile([NI, H], FP, tag="xi_row", name="xi_row")
        pxir = pep.tile([NI, H], FP, tag="ps", name="pxir")
        nc.tensor.transpose(pxir, xiT_sb, pk["identp"])
        nc.vector.tensor_copy(xi_row, pxir)

        y_sb = work.tile([NI, H], FP, tag="y_sb", name="y_sb")
        rowsum = work.tile([NI, 1], FP, tag="rowsum", name="rowsum")
        nc.vector.scalar_tensor_tensor(out=y_sb, in0=py, scalar=0.0,
                                       in1=xi_row, op0=ALU.add, op1=ALU.add,
                                       accum_out=rowsum)
        negmu = work.tile([NI, 1], FP, tag="negmu", name="negmu")
        nc.vector.tensor_scalar(negmu, rowsum, -1.0 / H, None, ALU.mult)

        ysq = work.tile([NI, H], FP, tag="ysq", name="ysq")
        sumsq = work.tile([NI, 1], FP, tag="sumsq", name="sumsq")
        nc.vector.scalar_tensor_tensor(out=ysq, in0=y_sb, scalar=0.0,
                                       in1=y_sb, op0=ALU.add, op1=ALU.mult,
                                       accum_out=sumsq)
        ex2 = work.tile([NI, 1], FP, tag="ex2", name="ex2")
        nc.vector.tensor_scalar(ex2, sumsq, 1.0 / H, float(EPS),
                                ALU.mult, ALU.add)
        musq = work.tile([NI, 1], FP, tag="musq", name="musq")
        nc.vector.scalar_tensor_tensor(out=musq, in0=negmu, scalar=0.0,
                                       in1=negmu, op0=ALU.add, op1=ALU.mult)
        vare = work.tile([NI, 1], FP, tag="vare", name="vare")
        nc.vector.scalar_tensor_tensor(out=vare, in0=ex2, scalar=0.0,
                                       in1=musq, op0=ALU.add,
                                       op1=ALU.subtract)
        sd = work.tile([NI, 1], FP, tag="sd", name="sd")
        nc.scalar.activation(sd, vare, ACTF.Sqrt)
        rstd = work.tile([NI, 1], FP, tag="rstd", name="rstd")
        nc.vector.reciprocal(rstd, sd)

        yn = work.tile([NI, H], FP, tag="yn", name="yn")
        nc.vector.tensor_scalar(yn, y_sb, negmu, rstd, ALU.add, ALU.mult)
        yg = work.tile([NI, H], FP, tag="yg", name="yg")
        nc.vector.scalar_tensor_tensor(out=yg, in0=yn, scalar=0.0,
                                       in1=pk["gamma_rep"], op0=ALU.add,
                                       op1=ALU.mult)
        yfin = work.tile([NI, H], FP, tag="yfin", name="yfin")
        nc.vector.scalar_tensor_tensor(out=yfin, in0=yg, scalar=0.0,
                                       in1=pk["beta_rep"], op0=ALU.add,
                                       op1=ALU.add)
        nc.sync.dma_start(out=out[:], in_=yfin)

    nc.finalize()
    return nc


def _get_program(NJC):
    key = ("nc", NJC)
    if key not in _cache:
        _cache[key] = _build_program(NJC)
    return _cache[key]


def kernel(x, adj_dist, mask, cond_vec, W1, b1, W2, b2, W3, b3, W4, b4,
           gamma, beta):
    x = np.asarray(x, dtype=np.float32)
    adj_dist = np.asarray(adj_dist, dtype=np.float32)
    mask_np = np.asarray(mask)
    cond_vec = np.asarray(cond_vec, dtype=np.float32)
    W1 = np.asarray(W1, dtype=np.float32)
    W2 = np.asarray(W2, dtype=np.float32)
    W3 = np.asarray(W3, dtype=np.float32)
    W4 = np.asarray(W4, dtype=np.float32)

    def c(a):
        return np.ascontiguousarray(a, dtype=np.float32)

    # j-axis compaction: live sender indices per batch, padded to common NJC
    live_idx = [np.where(mask_np[b] != 0)[0] for b in range(B)]
    NJC = max(8, -(-max(len(ix) for ix in live_idx) // 8) * 8)

    # DoubleRow lhsT: [H(k), 4(g), 2(ktile), H(m)] fp8
    w1bT = W1[:, H:2 * H].T               # (k, h)
    w1dT = W1[:, 2 * H:2 * H + R].T       # (r, h)
    lhs8 = np.zeros((H, 4, 2, H), dtype=np.float32)
    lhs8[:, :, 0, :] = (w1bT * SW)[:, None, :]
    for g in range(4):
        lhs8[32 * g:32 * g + 32, g, 1, :] = w1dT * SD
    lhs8 = lhs8.astype(ml_f8)

    rows_np = np.zeros((1, 6, H), dtype=np.float32)
    rows_np[0, 0] = np.asarray(b1)
    rows_np[0, 1] = np.asarray(b2)
    rows_np[0, 2] = np.asarray(b3)
    rows_np[0, 3] = np.asarray(b4)
    rows_np[0, 4] = 1.0
    rows_np[0, 5] = EPS

    rowsb_np = rows_np[:, :5].astype(ml_bf16)

    packb_np = np.stack([W2.T, W3[:, 0:H].T, W3[:, H:2 * H].T, W4.T],
                        axis=1).astype(ml_bf16)

    shared = dict(
        lhs8=lhs8,
        rows=rows_np,
        rowsb=rowsb_np,
        packb=np.ascontiguousarray(packb_np),
    )

    gamma_rep = np.tile(np.asarray(gamma, dtype=np.float32)[None, :], (H, 1))
    beta_rep = np.tile(np.asarray(beta, dtype=np.float32)[None, :], (H, 1))

    in_maps = []
    for core in range(8):
        b, ih = core // 2, core % 2
        i0 = ih * NI
        ix = live_idx[b]
        nlive = len(ix)

        # gathered x^T, masked (pad cols zero), scaled, fp8
        xg = np.zeros((H, NJC), dtype=np.float32)
        xg[:, :nlive] = x[b][ix].T / SW
        # adj stacks: [(g r), q, j] = adj[i0+4q+g, j_live, r] / SD
        ag = adj_dist[b, i0:i0 + NI][:, ix, :]          # (128, nlive, R)
        stk = np.zeros((H, NQ, NJC), dtype=np.float32)
        a4 = ag.reshape(NQ, 4, nlive, R)                # (q, g, j, r)
        stk[:, :, :nlive] = (a4.transpose(1, 3, 0, 2)   # (g, r, q, j)
                             .reshape(H, NQ, nlive)) / SD
        mf = np.zeros((NJC,), dtype=np.float32)
        mf[:nlive] = 1.0

        condrep = np.tile(cond_vec[b][:, None], (1, H)).astype(np.float32)
        packf_np = np.stack(
            [W1[:, 0:H].T, W1[:, 2 * H + R:3 * H + R].T,
             W1[:, 3 * H + R:].T, condrep[0:H], condrep[H:2 * H],
             np.eye(H, dtype=np.float32), gamma_rep, beta_rep], axis=1)

        m = dict(shared)
        m["adj_stk"] = stk.astype(ml_f8)
        m["xT8"] = xg.astype(ml_f8)
        m["xiT"] = c(x[b, i0:i0 + NI].T)
        m["maskf"] = mf
        m["packf"] = np.ascontiguousarray(packf_np)
        in_maps.append(m)

    nc = _get_program(NJC)
    _cache["in_maps"] = in_maps
    _cache["last_njc"] = NJC
    res = run_bass_kernel_spmd(nc, in_maps, list(range(8)))

    out_full = np.empty((B, N, H), dtype=np.float32)
    for core in range(8):
        b, ih = core // 2, core % 2
        out_full[b, ih * NI:(ih + 1) * NI] = res.results[core]["out"]
    return out_full


# revision 10
# speedup vs baseline: 2.3580x; 1.0186x over previous
"""CGNN layer kernel for Trainium2 (8 NeuronCores, SPMD) — v4.

Sharding: core c owns batch b = c//2 and receiver-node half i0 = (c%2)*128.

Host-side prep (layout only):
  - j-axis compaction: per batch, gather the live sender columns (mask==1)
    and pad to a common NJC (multiple of 8). Padded columns are zero; the
    on-device korr correction (which removes silu(bias) pollution from
    zeroed columns) covers them via the shipped 0/1 maskf.
  - adj is pre-transposed to the PE-ready stack layout
    stk[(g r), q, j] = adj[i0 + 4q + g, j, r], masked, scaled 1/SD, fp8.
  - x^T masked/scaled/fp8 for the x_j term; xi^T fp32 for the ACb term.
  - W1 is split and packed into 4 DoubleRow lhsT variants
    L_g = [w1bT*SW ; Z_g] fp8 where Z_g has W1dT*SD at partition band g.
  - all small fp32 [H,H] consts ride in ONE packed DRAM param (one DMA);
    bias rows in another.

Device math (per core, b fixed):
  z[i] (h=128, j=NJC) = ONE fp8 DoubleRow matmul:
      ktile0: (W1b*SW)^T @ (x^T*mask/SW)   [K=128]
      ktile1: Z_g^T @ stack_q              [K=128, band-selected adj term]
  silu + per-receiver bias ACb[:,i]: ONE ACT op -> bf16 sink slice;
  per-quad segmented DVE tensor_reduce sums 4 receivers over j at once.
  ACb = W1a x_i + W1c c + b1 (fp32 matmuls, setup).
  S -= npad_or_dead * silu(ACb); aggr = W2 S + b2*live; update MLP (bf16
  matmuls) + LayerNorm epilogue.
"""

import numpy as np
import ml_dtypes
from contextlib import ExitStack

import concourse.bass as bass
import concourse.bacc as bacc
import concourse.mybir as mybir
import concourse.tile as tile
from concourse.bass_utils import run_bass_kernel_spmd

ml_bf16 = ml_dtypes.bfloat16
ml_f8 = ml_dtypes.float8_e4m3

B, N, H, R = 4, 256, 128, 32
NI = 128          # receivers per core
NQ = NI // 4      # receiver quads
FP = mybir.dt.float32
BF = mybir.dt.bfloat16
F8 = mybir.dt.float8e4
EPS = 1e-5
ALU = mybir.AluOpType
ACTF = mybir.ActivationFunctionType
DR = mybir.MatmulPerfMode.DoubleRow

SW = 8.0   # fp8 scale for the W1b / x^T k-tile
SD = 8.0   # fp8 scale for the W1d / adj k-tile

# packed fp32 [H,H] const slots
PKC = ["w1aT", "w1cT0", "w1cT1", "condrep0", "condrep1"]
PKO = ["identp", "gamma_rep", "beta_rep"]
# packed bf16 [H,H] const slots (epilogue matmul weights)
PKB = ["w2T", "w3aT", "w3bT", "w4T"]

_cache = {}


def _build_program(NJC):
    nc = bacc.Bacc()

    # ---- per-core DRAM parameters ----
    adj_stk = nc.declare_dram_parameter("adj_stk", [H, NQ, NJC], F8,
                                        isOutput=False)
    xT8 = nc.declare_dram_parameter("xT8", [H, NJC], F8, isOutput=False)
    xiT = nc.declare_dram_parameter("xiT", [H, NI], FP, isOutput=False)
    maskf = nc.declare_dram_parameter("maskf", [NJC], FP, isOutput=False)
    lhs8 = nc.declare_dram_parameter("lhs8", [H, 4, 2, H], F8, isOutput=False)
    packc = nc.declare_dram_parameter("packc", [H, len(PKC), H], FP,
                                      isOutput=False)
    packo = nc.declare_dram_parameter("packo", [H, len(PKO), H], FP,
                                      isOutput=False)
    packb = nc.declare_dram_parameter("packb", [H, len(PKB), H], BF,
                                      isOutput=False)
    rows = nc.declare_dram_parameter("rows", [1, 6, H], FP, isOutput=False)
    rowsb = nc.declare_dram_parameter("rowsb", [1, 5, H], BF, isOutput=False)
    out = nc.declare_dram_parameter("out", [NI, H], FP, isOutput=True)

    with ExitStack() as ctx:
        tc = ctx.enter_context(tile.TileContext(nc))
        const = ctx.enter_context(tc.tile_pool(name="const", bufs=1))
        persist = ctx.enter_context(tc.tile_pool(name="persist", bufs=1))
        work = ctx.enter_context(tc.tile_pool(name="work", bufs=2))
        scr = ctx.enter_context(tc.tile_pool(name="scr", bufs=3))
        pep = ctx.enter_context(tc.tile_pool(name="pep", bufs=2,
                                             space="PSUM"))
        pz = ctx.enter_context(tc.tile_pool(name="pz", bufs=6, space="PSUM"))

        # rhs "big" tile: slot 0 = x^T (masked, /SW, fp8); slots 1..NQ = adj
        # stacks, DMA'd directly from host-prepped DRAM. Issue these FIRST
        # (they gate the PE main loop), split across queues.
        rhsbig = persist.tile([H, NQ + 1, NJC], F8, tag="rhsbig",
                              name="rhsbig")
        # ACb-critical loads first (small, gate the ACT pipeline)
        xiT_sb = const.tile([H, NI], FP, tag="xiT", name="xiT")
        nc.sync.dma_start(out=xiT_sb, in_=xiT[:])
        packc_sb = const.tile([H, len(PKC), H], FP, tag="packc",
                              name="packc")
        nc.scalar.dma_start(out=packc_sb, in_=packc[:])
        rows_sb = const.tile([1, 6, H], FP, tag="rows", name="rows")
        nc.scalar.dma_start(out=rows_sb, in_=rows[:])
        lhs8_sb = const.tile([H, 4, 2, H], F8, tag="lhs8", name="lhs8")
        nc.gpsimd.dma_start(out=lhs8_sb, in_=lhs8[:])

        nc.sync.dma_start(out=rhsbig[:, 0], in_=xT8[:])
        CH = NQ // 8
        for ci in range(8):
            eng = nc.sync if ci % 2 == 0 else nc.gpsimd
            eng.dma_start(
                out=rhsbig[:, 1 + ci * CH:1 + (ci + 1) * CH],
                in_=adj_stk[:, ci * CH:(ci + 1) * CH])

        packb_sb = const.tile([H, len(PKB), H], BF, tag="packb", name="packb")
        nc.sync.dma_start(out=packb_sb, in_=packb[:])
        rowsb_sb = const.tile([1, 5, H], BF, tag="rowsb", name="rowsb")
        nc.gpsimd.dma_start(out=rowsb_sb, in_=rowsb[:])

        packo_sb = const.tile([H, len(PKO), H], FP, tag="packo",
                              name="packo")
        nc.scalar.dma_start(out=packo_sb, in_=packo[:])
        pk = {name: packc_sb[:, i] for i, name in enumerate(PKC)}
        pk.update({name: packo_sb[:, i] for i, name in enumerate(PKO)})
        pkb = {name: packb_sb[:, i] for i, name in enumerate(PKB)}
        b1r = rows_sb[0:1, 0]
        ones_r = rows_sb[0:1, 4]
        eps_r = rows_sb[0:1, 5]
        b2rb = rowsb_sb[0:1, 1]
        b3rb = rowsb_sb[0:1, 2]
        b4rb = rowsb_sb[0:1, 3]
        onesb_r = rowsb_sb[0:1, 4]

        # mask broadcast to all partitions: (128, NJC)
        maskrep = persist.tile([H, NJC], FP, tag="maskrep", name="maskrep")
        maskf_ap = maskf[:]
        mask_bcast = bass.AP(tensor=maskf_ap.tensor, offset=maskf_ap.offset,
                             ap=[[0, H]] + list(maskf_ap.ap))
        nc.gpsimd.dma_start(out=maskrep, in_=mask_bcast)

        # Preload the Silu ACT table early (hidden under DMA) with a dummy
        # op on the eps row (already-loaded const).
        dummy = work.tile([1, H], FP, tag="dummy", name="dummy")
        nc.scalar.activation(dummy, eps_r, ACTF.Silu)

        # per-partition live-count and dead/pad-count of sender slots
        msum = persist.tile([H, 1], FP, tag="msum", name="msum")
        mrow_scr = persist.tile([H, NJC], FP, tag="mrow_scr", name="mrow_scr")
        nc.vector.tensor_scalar(mrow_scr, maskrep, 1.0, None,
                                ALU.mult, ALU.add, accum_out=msum)
        nm0col = persist.tile([H, 1], FP, tag="nm0col", name="nm0col")
        nc.vector.tensor_scalar(nm0col, msum, -1.0, float(NJC),
                                ALU.mult, ALU.add)
        msum_rowb = persist.tile([1, NI], BF, tag="msum_rowb",
                                 name="msum_rowb")
        nc.vector.tensor_scalar(msum_rowb, ones_r, msum[0:1, 0:1], None,
                                ALU.mult)

        ACb = persist.tile([H, NI], FP, tag="ACb", name="ACb")
        siluAC = persist.tile([H, NI], FP, tag="siluAC", name="siluAC")
        korr = persist.tile([H, NI], FP, tag="korr", name="korr")
        S_raw = persist.tile([H, NI], FP, tag="S_raw", name="S_raw")

        # ACb = W1a x_i + W1c c + b1  -> (128 h, 128 i)
        pA = pep.tile([H, NI], FP, tag="ps", name="pA")
        nc.tensor.matmul(pA, lhsT=pk["w1aT"], rhs=xiT_sb,
                         start=True, stop=False)
        nc.tensor.matmul(pA, lhsT=pk["w1cT0"], rhs=pk["condrep0"],
                         start=False, stop=False)
        nc.tensor.matmul(pA, lhsT=pk["w1cT1"], rhs=pk["condrep1"],
                         start=False, stop=False)
        nc.tensor.matmul(pA, lhsT=b1r, rhs=ones_r,
                         start=False, stop=True)
        nc.vector.tensor_copy(ACb, pA)

        # korr[h,i] = (dead+pad count) * silu(ACb[h,i])
        nc.scalar.activation(siluAC, ACb, ACTF.Silu)
        nc.vector.tensor_scalar(korr, siluAC, nm0col, None, ALU.mult)

        # x_i in row-major for the residual add (setup, off the tail path)
        xi_row = persist.tile([NI, H], FP, tag="xi_row", name="xi_row")
        pxir = pep.tile([NI, H], FP, tag="ps", name="pxir")
        nc.tensor.transpose(pxir, xiT_sb, pk["identp"])
        nc.vector.tensor_copy(xi_row, pxir)

        # ---- main loop: one DoubleRow matmul + one ACT per receiver;
        # one segmented DVE reduce (sum over j, 4 receivers) per quad ----
        for q in range(NQ):
            rhs_q = rhsbig[:, 0:q + 2:q + 1]   # slots {0, q+1}
            sinkq = scr.tile([H, 4, NJC], BF, tag="sinkq", name="sinkq")
            for g in range(4):
                li = 4 * q + g
                zt = pz.tile([H, NJC], FP, tag="zt", name="zt")
                nc.tensor.matmul(zt, lhsT=lhs8_sb[:, g], rhs=rhs_q,
                                 start=True, stop=True, perf_mode=DR)
                nc.scalar.activation(sinkq[:, g], zt, ACTF.Silu,
                                     bias=ACb[:, li:li + 1])
            nc.vector.tensor_reduce(S_raw[:, 4 * q:4 * q + 4], sinkq,
                                    mybir.AxisListType.X, ALU.add)

        # ---- epilogue ----
        S_true = persist.tile([H, NI], BF, tag="S_true", name="S_true")
        nc.vector.scalar_tensor_tensor(out=S_true, in0=S_raw, scalar=0.0,
                                       in1=korr, op0=ALU.add,
                                       op1=ALU.subtract)
        # aggr = W2 s + b2 * live_count
        pa = pep.tile([H, NI], FP, tag="ps", name="pa")
        nc.tensor.matmul(pa, lhsT=pkb["w2T"], rhs=S_true,
                         start=True, stop=False)
        nc.tensor.matmul(pa, lhsT=b2rb, rhs=msum_rowb, start=False,
                         stop=True)
        aggrT = work.tile([H, NI], BF, tag="aggrT", name="aggrT")
        nc.vector.tensor_copy(aggrT, pa)
        xiTb = work.tile([H, NI], BF, tag="xiTb", name="xiTb")
        nc.gpsimd.tensor_copy(xiTb, xiT_sb)
        onesNIb = rowsb_sb[0:1, 4]   # bf16 ones row (NI == H)

        pu = pep.tile([H, NI], FP, tag="ps", name="pu")
        nc.tensor.matmul(pu, lhsT=pkb["w3aT"], rhs=xiTb,
                         start=True, stop=False)
        nc.tensor.matmul(pu, lhsT=pkb["w3bT"], rhs=aggrT,
                         start=False, stop=False)
        nc.tensor.matmul(pu, lhsT=b3rb, rhs=onesNIb, start=False,
                         stop=True)
        u_sb = work.tile([H, NI], BF, tag="u_sb", name="u_sb")
        nc.scalar.activation(u_sb, pu, ACTF.Silu)

        pupd = pep.tile([H, NI], FP, tag="ps", name="pupd")
        nc.tensor.matmul(pupd, lhsT=pkb["w4T"], rhs=u_sb,
                         start=True, stop=False)
        nc.tensor.matmul(pupd, lhsT=b4rb, rhs=onesNIb, start=False,
                         stop=True)
        updT = work.tile([H, NI], FP, tag="updT", name="updT")
        nc.vector.tensor_copy(updT, pupd)

        py = pep.tile([NI, H], FP, tag="ps", name="py")
        nc.tensor.transpose(py, updT, pk["identp"])

        y_sb = work.tile([NI, H], FP, tag="y_sb", name="y_sb")
        rowsum = work.tile([NI, 1], FP, tag="rowsum", name="rowsum")
        nc.vector.scalar_tensor_tensor(out=y_sb, in0=py, scalar=0.0,
                                       in1=xi_row, op0=ALU.add, op1=ALU.add,
                                       accum_out=rowsum)
        negmu = work.tile([NI, 1], FP, tag="negmu", name="negmu")
        nc.vector.tensor_scalar(negmu, rowsum, -1.0 / H, None, ALU.mult)

        ysq = work.tile([NI, H], FP, tag="ysq", name="ysq")
        sumsq = work.tile([NI, 1], FP, tag="sumsq", name="sumsq")
        nc.vector.scalar_tensor_tensor(out=ysq, in0=y_sb, scalar=0.0,
                                       in1=y_sb, op0=ALU.add, op1=ALU.mult,
                                       accum_out=sumsq)
        ex2 = work.tile([NI, 1], FP, tag="ex2", name="ex2")
        nc.vector.tensor_scalar(ex2, sumsq, 1.0 / H, float(EPS),
                                ALU.mult, ALU.add)
        musq = work.tile([NI, 1], FP, tag="musq", name="musq")
        nc.vector.scalar_tensor_tensor(out=musq, in0=negmu, scalar=0.0,
                                       in1=negmu, op0=ALU.add, op1=ALU.mult)
        vare = work.tile([NI, 1], FP, tag="vare", name="vare")
        nc.vector.scalar_tensor_tensor(out=vare, in0=ex2, scalar=0.0,
                                       in1=musq, op0=ALU.add,
                                       op1=ALU.subtract)
        sd = work.tile([NI, 1], FP, tag="sd", name="sd")
        nc.scalar.activation(sd, vare, ACTF.Sqrt)
        rstd = work.tile([NI, 1], FP, tag="rstd", name="rstd")
        nc.vector.reciprocal(rstd, sd)

        yn = work.tile([NI, H], FP, tag="yn", name="yn")
        nc.vector.tensor_scalar(yn, y_sb, negmu, rstd, ALU.add, ALU.mult)
        yg = work.tile([NI, H], FP, tag="yg", name="yg")
        nc.vector.scalar_tensor_tensor(out=yg, in0=yn, scalar=0.0,
                                       in1=pk["gamma_rep"], op0=ALU.add,
                                       op1=ALU.mult)
        yfin = work.tile([NI, H], FP, tag="yfin", name="yfin")
        nc.vector.scalar_tensor_tensor(out=yfin, in0=yg, scalar=0.0,
                                       in1=pk["beta_rep"], op0=ALU.add,
                                       op1=ALU.add)
        nc.sync.dma_start(out=out[:], in_=yfin)

    nc.finalize()
    return nc


def _get_program(NJC):
    key = ("nc", NJC)
    if key not in _cache:
        _cache[key] = _build_program(NJC)
    return _cache[key]


def kernel(x, adj_dist, mask, cond_vec, W1, b1, W2, b2, W3, b3, W4, b4,
           gamma, beta):
    x = np.asarray(x, dtype=np.float32)
    adj_dist = np.asarray(adj_dist, dtype=np.float32)
    mask_np = np.asarray(mask)
    cond_vec = np.asarray(cond_vec, dtype=np.float32)
    W1 = np.asarray(W1, dtype=np.float32)
    W2 = np.asarray(W2, dtype=np.float32)
    W3 = np.asarray(W3, dtype=np.float32)
    W4 = np.asarray(W4, dtype=np.float32)

    def c(a):
        return np.ascontiguousarray(a, dtype=np.float32)

    # j-axis compaction: live sender indices per batch, padded to common NJC
    live_idx = [np.where(mask_np[b] != 0)[0] for b in range(B)]
    NJC = max(8, -(-max(len(ix) for ix in live_idx) // 8) * 8)

    # DoubleRow lhsT: [H(k), 4(g), 2(ktile), H(m)] fp8
    w1bT = W1[:, H:2 * H].T               # (k, h)
    w1dT = W1[:, 2 * H:2 * H + R].T       # (r, h)
    lhs8 = np.zeros((H, 4, 2, H), dtype=np.float32)
    lhs8[:, :, 0, :] = (w1bT * SW)[:, None, :]
    for g in range(4):
        lhs8[32 * g:32 * g + 32, g, 1, :] = w1dT * SD
    lhs8 = lhs8.astype(ml_f8)

    rows_np = np.zeros((1, 6, H), dtype=np.float32)
    rows_np[0, 0] = np.asarray(b1)
    rows_np[0, 1] = np.asarray(b2)
    rows_np[0, 2] = np.asarray(b3)
    rows_np[0, 3] = np.asarray(b4)
    rows_np[0, 4] = 1.0
    rows_np[0, 5] = EPS

    rowsb_np = rows_np[:, :5].astype(ml_bf16)

    packb_np = np.stack([W2.T, W3[:, 0:H].T, W3[:, H:2 * H].T, W4.T],
                        axis=1).astype(ml_bf16)

    gamma_rep = np.tile(np.asarray(gamma, dtype=np.float32)[None, :], (H, 1))
    beta_rep = np.tile(np.asarray(beta, dtype=np.float32)[None, :], (H, 1))
    packo_np = np.stack([np.eye(H, dtype=np.float32), gamma_rep, beta_rep],
                        axis=1)
    shared = dict(
        packo=np.ascontiguousarray(packo_np),
        lhs8=lhs8,
        rows=rows_np,
        rowsb=rowsb_np,
        packb=np.ascontiguousarray(packb_np),
    )


    in_maps = []
    for core in range(8):
        b, ih = core // 2, core % 2
        i0 = ih * NI
        ix = live_idx[b]
        nlive = len(ix)

        # gathered x^T, masked (pad cols zero), scaled, fp8
        xg = np.zeros((H, NJC), dtype=np.float32)
        xg[:, :nlive] = x[b][ix].T / SW
        # adj stacks: [(g r), q, j] = adj[i0+4q+g, j_live, r] / SD
        ag = adj_dist[b, i0:i0 + NI][:, ix, :]          # (128, nlive, R)
        stk = np.zeros((H, NQ, NJC), dtype=np.float32)
        a4 = ag.reshape(NQ, 4, nlive, R)                # (q, g, j, r)
        stk[:, :, :nlive] = (a4.transpose(1, 3, 0, 2)   # (g, r, q, j)
                             .reshape(H, NQ, nlive)) / SD
        mf = np.zeros((NJC,), dtype=np.float32)
        mf[:nlive] = 1.0

        condrep = np.tile(cond_vec[b][:, None], (1, H)).astype(np.float32)
        packc_np = np.stack(
            [W1[:, 0:H].T, W1[:, 2 * H + R:3 * H + R].T,
             W1[:, 3 * H + R:].T, condrep[0:H], condrep[H:2 * H]], axis=1)

        m = dict(shared)
        m["adj_stk"] = stk.astype(ml_f8)
        m["xT8"] = xg.astype(ml_f8)
        m["xiT"] = c(x[b, i0:i0 + NI].T)
        m["maskf"] = mf
        m["packc"] = np.ascontiguousarray(packc_np)
        in_maps.append(m)

    nc = _get_program(NJC)
    _cache["in_maps"] = in_maps
    _cache["last_njc"] = NJC
    res = run_bass_kernel_spmd(nc, in_maps, list(range(8)))

    out_full = np.empty((B, N, H), dtype=np.float32)
    for core in range(8):
        b, ih = core // 2, core % 2
        out_full[b, ih * NI:(ih + 1) * NI] = res.results[core]["out"]
    return out_full


# revision 11
# speedup vs baseline: 2.3704x; 1.0053x over previous
"""CGNN layer kernel for Trainium2 (8 NeuronCores, SPMD) — v4.

Sharding: core c owns batch b = c//2 and receiver-node half i0 = (c%2)*128.

Host-side prep (layout only):
  - j-axis compaction: per batch, gather the live sender columns (mask==1)
    and pad to a common NJC (multiple of 8). Padded columns are zero; the
    on-device korr correction (which removes silu(bias) pollution from
    zeroed columns) covers them via the shipped 0/1 maskf.
  - adj is pre-transposed to the PE-ready stack layout
    stk[(g r), q, j] = adj[i0 + 4q + g, j, r], masked, scaled 1/SD, fp8.
  - x^T masked/scaled/fp8 for the x_j term; xi^T fp32 for the ACb term.
  - W1 is split and packed into 4 DoubleRow lhsT variants
    L_g = [w1bT*SW ; Z_g] fp8 where Z_g has W1dT*SD at partition band g.
  - all small fp32 [H,H] consts ride in ONE packed DRAM param (one DMA);
    bias rows in another.

Device math (per core, b fixed):
  z[i] (h=128, j=NJC) = ONE fp8 DoubleRow matmul:
      ktile0: (W1b*SW)^T @ (x^T*mask/SW)   [K=128]
      ktile1: Z_g^T @ stack_q              [K=128, band-selected adj term]
  silu + per-receiver bias ACb[:,i]: ONE ACT op -> bf16 sink slice;
  per-quad segmented DVE tensor_reduce sums 4 receivers over j at once.
  ACb = W1a x_i + W1c c + b1 (fp32 matmuls, setup).
  S -= npad_or_dead * silu(ACb); aggr = W2 S + b2*live; update MLP (bf16
  matmuls) + LayerNorm epilogue.
"""

import numpy as np
import ml_dtypes
from contextlib import ExitStack

import concourse.bass as bass
import concourse.bacc as bacc
import concourse.mybir as mybir
import concourse.tile as tile
from concourse.bass_utils import run_bass_kernel_spmd

ml_bf16 = ml_dtypes.bfloat16
ml_f8 = ml_dtypes.float8_e4m3

B, N, H, R = 4, 256, 128, 32
NI = 128          # receivers per core
NQ = NI // 4      # receiver quads
FP = mybir.dt.float32
BF = mybir.dt.bfloat16
F8 = mybir.dt.float8e4
EPS = 1e-5
ALU = mybir.AluOpType
ACTF = mybir.ActivationFunctionType
DR = mybir.MatmulPerfMode.DoubleRow

SW = 8.0   # fp8 scale for the W1b / x^T k-tile
SD = 8.0   # fp8 scale for the W1d / adj k-tile

# packed fp32 [H,H] const slots
PKC = ["w1aT", "w1cT0", "w1cT1", "condrep0", "condrep1"]
PKO = ["identp", "gamma_rep", "beta_rep"]
# packed bf16 [H,H] const slots (epilogue matmul weights)
PKB = ["w2T", "w3aT", "w3bT", "w4T"]

_cache = {}


def _build_program(NJC):
    nc = bacc.Bacc()

    # ---- per-core DRAM parameters ----
    adj_stk = nc.declare_dram_parameter("adj_stk", [H, NQ, NJC], F8,
                                        isOutput=False)
    xT8 = nc.declare_dram_parameter("xT8", [H, NJC], F8, isOutput=False)
    xiT = nc.declare_dram_parameter("xiT", [H, NI], FP, isOutput=False)
    maskf = nc.declare_dram_parameter("maskf", [NJC], FP, isOutput=False)
    lhs8 = nc.declare_dram_parameter("lhs8", [H, 4, 2, H], F8, isOutput=False)
    packc = nc.declare_dram_parameter("packc", [H, len(PKC), H], FP,
                                      isOutput=False)
    packo = nc.declare_dram_parameter("packo", [H, len(PKO), H], FP,
                                      isOutput=False)
    packb = nc.declare_dram_parameter("packb", [H, len(PKB), H], BF,
                                      isOutput=False)
    rows = nc.declare_dram_parameter("rows", [1, 6, H], FP, isOutput=False)
    rowsb = nc.declare_dram_parameter("rowsb", [1, 5, H], BF, isOutput=False)
    out = nc.declare_dram_parameter("out", [NI, H], FP, isOutput=True)

    with ExitStack() as ctx:
        tc = ctx.enter_context(tile.TileContext(nc))
        const = ctx.enter_context(tc.tile_pool(name="const", bufs=1))
        persist = ctx.enter_context(tc.tile_pool(name="persist", bufs=1))
        work = ctx.enter_context(tc.tile_pool(name="work", bufs=2))
        scr = ctx.enter_context(tc.tile_pool(name="scr", bufs=4))
        pep = ctx.enter_context(tc.tile_pool(name="pep", bufs=2,
                                             space="PSUM"))
        pz = ctx.enter_context(tc.tile_pool(name="pz", bufs=6, space="PSUM"))

        # rhs "big" tile: slot 0 = x^T (masked, /SW, fp8); slots 1..NQ = adj
        # stacks, DMA'd directly from host-prepped DRAM. Issue these FIRST
        # (they gate the PE main loop), split across queues.
        rhsbig = persist.tile([H, NQ + 1, NJC], F8, tag="rhsbig",
                              name="rhsbig")
        # ACb-critical loads first (small, gate the ACT pipeline)
        xiT_sb = const.tile([H, NI], FP, tag="xiT", name="xiT")
        nc.sync.dma_start(out=xiT_sb, in_=xiT[:])
        packc_sb = const.tile([H, len(PKC), H], FP, tag="packc",
                              name="packc")
        nc.scalar.dma_start(out=packc_sb, in_=packc[:])
        rows_sb = const.tile([1, 6, H], FP, tag="rows", name="rows")
        nc.scalar.dma_start(out=rows_sb, in_=rows[:])
        lhs8_sb = const.tile([H, 4, 2, H], F8, tag="lhs8", name="lhs8")
        nc.gpsimd.dma_start(out=lhs8_sb, in_=lhs8[:])

        nc.sync.dma_start(out=rhsbig[:, 0], in_=xT8[:])
        CH = NQ // 8
        for ci in range(8):
            eng = nc.sync if ci % 2 == 0 else nc.gpsimd
            eng.dma_start(
                out=rhsbig[:, 1 + ci * CH:1 + (ci + 1) * CH],
                in_=adj_stk[:, ci * CH:(ci + 1) * CH])

        packb_sb = const.tile([H, len(PKB), H], BF, tag="packb", name="packb")
        nc.sync.dma_start(out=packb_sb, in_=packb[:])
        rowsb_sb = const.tile([1, 5, H], BF, tag="rowsb", name="rowsb")
        nc.gpsimd.dma_start(out=rowsb_sb, in_=rowsb[:])

        packo_sb = const.tile([H, len(PKO), H], FP, tag="packo",
                              name="packo")
        nc.scalar.dma_start(out=packo_sb, in_=packo[:])
        pk = {name: packc_sb[:, i] for i, name in enumerate(PKC)}
        pk.update({name: packo_sb[:, i] for i, name in enumerate(PKO)})
        pkb = {name: packb_sb[:, i] for i, name in enumerate(PKB)}
        b1r = rows_sb[0:1, 0]
        ones_r = rows_sb[0:1, 4]
        eps_r = rows_sb[0:1, 5]
        b2rb = rowsb_sb[0:1, 1]
        b3rb = rowsb_sb[0:1, 2]
        b4rb = rowsb_sb[0:1, 3]
        onesb_r = rowsb_sb[0:1, 4]

        # mask broadcast to all partitions: (128, NJC)
        maskrep = persist.tile([H, NJC], FP, tag="maskrep", name="maskrep")
        maskf_ap = maskf[:]
        mask_bcast = bass.AP(tensor=maskf_ap.tensor, offset=maskf_ap.offset,
                             ap=[[0, H]] + list(maskf_ap.ap))
        nc.gpsimd.dma_start(out=maskrep, in_=mask_bcast)

        # Preload the Silu ACT table early (hidden under DMA) with a dummy
        # op on the eps row (already-loaded const).
        dummy = work.tile([1, H], FP, tag="dummy", name="dummy")
        nc.scalar.activation(dummy, eps_r, ACTF.Silu)

        # per-partition live-count and dead/pad-count of sender slots
        msum = persist.tile([H, 1], FP, tag="msum", name="msum")
        mrow_scr = persist.tile([H, NJC], FP, tag="mrow_scr", name="mrow_scr")
        nc.vector.tensor_scalar(mrow_scr, maskrep, 1.0, None,
                                ALU.mult, ALU.add, accum_out=msum)
        nm0col = persist.tile([H, 1], FP, tag="nm0col", name="nm0col")
        nc.vector.tensor_scalar(nm0col, msum, -1.0, float(NJC),
                                ALU.mult, ALU.add)
        msum_rowb = persist.tile([1, NI], BF, tag="msum_rowb",
                                 name="msum_rowb")
        nc.vector.tensor_scalar(msum_rowb, ones_r, msum[0:1, 0:1], None,
                                ALU.mult)

        ACb = persist.tile([H, NI], FP, tag="ACb", name="ACb")
        siluAC = persist.tile([H, NI], FP, tag="siluAC", name="siluAC")
        korr = persist.tile([H, NI], FP, tag="korr", name="korr")
        S_raw = persist.tile([H, NI], FP, tag="S_raw", name="S_raw")

        # ACb = W1a x_i + W1c c + b1  -> (128 h, 128 i)
        pA = pep.tile([H, NI], FP, tag="ps", name="pA")
        nc.tensor.matmul(pA, lhsT=pk["w1aT"], rhs=xiT_sb,
                         start=True, stop=False)
        nc.tensor.matmul(pA, lhsT=pk["w1cT0"], rhs=pk["condrep0"],
                         start=False, stop=False)
        nc.tensor.matmul(pA, lhsT=pk["w1cT1"], rhs=pk["condrep1"],
                         start=False, stop=False)
        nc.tensor.matmul(pA, lhsT=b1r, rhs=ones_r,
                         start=False, stop=True)
        nc.vector.tensor_copy(ACb, pA)

        # korr[h,i] = (dead+pad count) * silu(ACb[h,i])
        nc.scalar.activation(siluAC, ACb, ACTF.Silu)
        nc.vector.tensor_scalar(korr, siluAC, nm0col, None, ALU.mult)

        # x_i in row-major for the residual add (setup, off the tail path)
        xi_row = persist.tile([NI, H], FP, tag="xi_row", name="xi_row")
        pxir = pep.tile([NI, H], FP, tag="ps", name="pxir")
        nc.tensor.transpose(pxir, xiT_sb, pk["identp"])
        nc.vector.tensor_copy(xi_row, pxir)

        # ---- main loop: one DoubleRow matmul + one ACT per receiver;
        # one segmented DVE reduce (sum over j, 4 receivers) per quad ----
        for q in range(NQ):
            rhs_q = rhsbig[:, 0:q + 2:q + 1]   # slots {0, q+1}
            sinkq = scr.tile([H, 4, NJC], BF, tag="sinkq", name="sinkq")
            for g in range(4):
                li = 4 * q + g
                zt = pz.tile([H, NJC], FP, tag="zt", name="zt")
                nc.tensor.matmul(zt, lhsT=lhs8_sb[:, g], rhs=rhs_q,
                                 start=True, stop=True, perf_mode=DR)
                nc.scalar.activation(sinkq[:, g], zt, ACTF.Silu,
                                     bias=ACb[:, li:li + 1])
            nc.vector.tensor_reduce(S_raw[:, 4 * q:4 * q + 4], sinkq,
                                    mybir.AxisListType.X, ALU.add)

        # ---- epilogue ----
        S_true = persist.tile([H, NI], BF, tag="S_true", name="S_true")
        nc.vector.scalar_tensor_tensor(out=S_true, in0=S_raw, scalar=0.0,
                                       in1=korr, op0=ALU.add,
                                       op1=ALU.subtract)
        # aggr = W2 s + b2 * live_count
        pa = pep.tile([H, NI], FP, tag="ps", name="pa")
        nc.tensor.matmul(pa, lhsT=pkb["w2T"], rhs=S_true,
                         start=True, stop=False)
        nc.tensor.matmul(pa, lhsT=b2rb, rhs=msum_rowb, start=False,
                         stop=True)
        aggrT = work.tile([H, NI], BF, tag="aggrT", name="aggrT")
        nc.vector.tensor_copy(aggrT, pa)
        xiTb = work.tile([H, NI], BF, tag="xiTb", name="xiTb")
        nc.gpsimd.tensor_copy(xiTb, xiT_sb)
        onesNIb = rowsb_sb[0:1, 4]   # bf16 ones row (NI == H)

        pu = pep.tile([H, NI], FP, tag="ps", name="pu")
        nc.tensor.matmul(pu, lhsT=pkb["w3aT"], rhs=xiTb,
                         start=True, stop=False)
        nc.tensor.matmul(pu, lhsT=pkb["w3bT"], rhs=aggrT,
                         start=False, stop=False)
        nc.tensor.matmul(pu, lhsT=b3rb, rhs=onesNIb, start=False,
                         stop=True)
        u_sb = work.tile([H, NI], BF, tag="u_sb", name="u_sb")
        nc.scalar.activation(u_sb, pu, ACTF.Silu)

        pupd = pep.tile([H, NI], FP, tag="ps", name="pupd")
        nc.tensor.matmul(pupd, lhsT=pkb["w4T"], rhs=u_sb,
                         start=True, stop=False)
        nc.tensor.matmul(pupd, lhsT=b4rb, rhs=onesNIb, start=False,
                         stop=True)
        updT = work.tile([H, NI], FP, tag="updT", name="updT")
        nc.vector.tensor_copy(updT, pupd)

        py = pep.tile([NI, H], FP, tag="ps", name="py")
        nc.tensor.transpose(py, updT, pk["identp"])

        y_sb = work.tile([NI, H], FP, tag="y_sb", name="y_sb")
        rowsum = work.tile([NI, 1], FP, tag="rowsum", name="rowsum")
        nc.vector.scalar_tensor_tensor(out=y_sb, in0=py, scalar=0.0,
                                       in1=xi_row, op0=ALU.add, op1=ALU.add,
                                       accum_out=rowsum)
        negmu = work.tile([NI, 1], FP, tag="negmu", name="negmu")
        nc.vector.tensor_scalar(negmu, rowsum, -1.0 / H, None, ALU.mult)

        ysq = work.tile([NI, H], FP, tag="ysq", name="ysq")
        sumsq = work.tile([NI, 1], FP, tag="sumsq", name="sumsq")
        nc.vector.scalar_tensor_tensor(out=ysq, in0=y_sb, scalar=0.0,
                                       in1=y_sb, op0=ALU.add, op1=ALU.mult,
                                       accum_out=sumsq)
        ex2 = work.tile([NI, 1], FP, tag="ex2", name="ex2")
        nc.vector.tensor_scalar(ex2, sumsq, 1.0 / H, float(EPS),
                                ALU.mult, ALU.add)
        musq = work.tile([NI, 1], FP, tag="musq", name="musq")
        nc.vector.scalar_tensor_tensor(out=musq, in0=negmu, scalar=0.0,
                                       in1=negmu, op0=ALU.add, op1=ALU.mult)
        vare = work.tile([NI, 1], FP, tag="vare", name="vare")
        nc.vector.scalar_tensor_tensor(out=vare, in0=ex2, scalar=0.0,
                                       in1=musq, op0=ALU.add,
                                       op1=ALU.subtract)
        sd = work.tile([NI, 1], FP, tag="sd", name="sd")
        nc.scalar.activation(sd, vare, ACTF.Sqrt)
        rstd = work.tile([NI, 1], FP, tag="rstd", name="rstd")
        nc.vector.reciprocal(rstd, sd)

        yn = work.tile([NI, H], FP, tag="yn", name="yn")
        nc.vector.tensor_scalar(yn, y_sb, negmu, rstd, ALU.add, ALU.mult)
        yg = work.tile([NI, H], FP, tag="yg", name="yg")
        nc.vector.scalar_tensor_tensor(out=yg, in0=yn, scalar=0.0,
                                       in1=pk["gamma_rep"], op0=ALU.add,
                                       op1=ALU.mult)
        yfin = work.tile([NI, H], FP, tag="yfin", name="yfin")
        nc.vector.scalar_tensor_tensor(out=yfin, in0=yg, scalar=0.0,
                                       in1=pk["beta_rep"], op0=ALU.add,
                                       op1=ALU.add)
        nc.sync.dma_start(out=out[:], in_=yfin)

    nc.finalize()
    return nc


def _get_program(NJC):
    key = ("nc", NJC)
    if key not in _cache:
        _cache[key] = _build_program(NJC)
    return _cache[key]


def kernel(x, adj_dist, mask, cond_vec, W1, b1, W2, b2, W3, b3, W4, b4,
           gamma, beta):
    x = np.asarray(x, dtype=np.float32)
    adj_dist = np.asarray(adj_dist, dtype=np.float32)
    mask_np = np.asarray(mask)
    cond_vec = np.asarray(cond_vec, dtype=np.float32)
    W1 = np.asarray(W1, dtype=np.float32)
    W2 = np.asarray(W2, dtype=np.float32)
    W3 = np.asarray(W3, dtype=np.float32)
    W4 = np.asarray(W4, dtype=np.float32)

    def c(a):
        return np.ascontiguousarray(a, dtype=np.float32)

    # j-axis compaction: live sender indices per batch, padded to common NJC
    live_idx = [np.where(mask_np[b] != 0)[0] for b in range(B)]
    NJC = max(8, -(-max(len(ix) for ix in live_idx) // 8) * 8)

    # DoubleRow lhsT: [H(k), 4(g), 2(ktile), H(m)] fp8
    w1bT = W1[:, H:2 * H].T               # (k, h)
    w1dT = W1[:, 2 * H:2 * H + R].T       # (r, h)
    lhs8 = np.zeros((H, 4, 2, H), dtype=np.float32)
    lhs8[:, :, 0, :] = (w1bT * SW)[:, None, :]
    for g in range(4):
        lhs8[32 * g:32 * g + 32, g, 1, :] = w1dT * SD
    lhs8 = lhs8.astype(ml_f8)

    rows_np = np.zeros((1, 6, H), dtype=np.float32)
    rows_np[0, 0] = np.asarray(b1)
    rows_np[0, 1] = np.asarray(b2)
    rows_np[0, 2] = np.asarray(b3)
    rows_np[0, 3] = np.asarray(b4)
    rows_np[0, 4] = 1.0
    rows_np[0, 5] = EPS

    rowsb_np = rows_np[:, :5].astype(ml_bf16)

    packb_np = np.stack([W2.T, W3[:, 0:H].T, W3[:, H:2 * H].T, W4.T],
                        axis=1).astype(ml_bf16)

    gamma_rep = np.tile(np.asarray(gamma, dtype=np.float32)[None, :], (H, 1))
    beta_rep = np.tile(np.asarray(beta, dtype=np.float32)[None, :], (H, 1))
    packo_np = np.stack([np.eye(H, dtype=np.float32), gamma_rep, beta_rep],
                        axis=1)
    shared = dict(
        packo=np.ascontiguousarray(packo_np),
        lhs8=lhs8,
        rows=rows_np,
        rowsb=rowsb_np,
        packb=np.ascontiguousarray(packb_np),
    )


    in_maps = []
    for core in range(8):
        b, ih = core // 2, core % 2
        i0 = ih * NI
        ix = live_idx[b]
        nlive = len(ix)

        # gathered x^T, masked (pad cols zero), scaled, fp8
        xg = np.zeros((H, NJC), dtype=np.float32)
        xg[:, :nlive] = x[b][ix].T / SW
        # adj stacks: [(g r), q, j] = adj[i0+4q+g, j_live, r] / SD
        ag = adj_dist[b, i0:i0 + NI][:, ix, :]          # (128, nlive, R)
        stk = np.zeros((H, NQ, NJC), dtype=np.float32)
        a4 = ag.reshape(NQ, 4, nlive, R)                # (q, g, j, r)
        stk[:, :, :nlive] = (a4.transpose(1, 3, 0, 2)   # (g, r, q, j)
                             .reshape(H, NQ, nlive)) / SD
        mf = np.zeros((NJC,), dtype=np.float32)
        mf[:nlive] = 1.0

        condrep = np.tile(cond_vec[b][:, None], (1, H)).astype(np.float32)
        packc_np = np.stack(
            [W1[:, 0:H].T, W1[:, 2 * H + R:3 * H + R].T,
             W1[:, 3 * H + R:].T, condrep[0:H], condrep[H:2 * H]], axis=1)

        m = dict(shared)
        m["adj_stk"] = stk.astype(ml_f8)
        m["xT8"] = xg.astype(ml_f8)
        m["xiT"] = c(x[b, i0:i0 + NI].T)
        m["maskf"] = mf
        m["packc"] = np.ascontiguousarray(packc_np)
        in_maps.append(m)

    nc = _get_program(NJC)
    _cache["in_maps"] = in_maps
    _cache["last_njc"] = NJC
    res = run_bass_kernel_spmd(nc, in_maps, list(range(8)))

    out_full = np.empty((B, N, H), dtype=np.float32)
    for core in range(8):
        b, ih = core // 2, core % 2
        out_full[b, ih * NI:(ih + 1) * NI] = res.results[core]["out"]
    return out_full
